# revision 24
# baseline (speedup 1.0000x reference)
"""MoE batched-experts kernel for Trainium2 (8 NeuronCores, expert-parallel).

Contract: kernel(**inputs) takes the FULL unsharded inputs
  x:              [T, D]      float32   (T=16384, D=1024)
  expert_indices: [T]         int32     (values in [0, 8))
  gate_up_weight: [E, 2F, D]  float32   (E=8, F=2048)
  down_weight:    [E, D, F]   float32
and returns the FULL output [T, D] float32:
  per token t with expert e: h = silu(x @ gu[e,:F].T) * (x @ gu[e,F:].T);
  out = h @ dw[e].T

Strategy: expert-parallel. The host routes (sorts) tokens by expert, pads
each expert's group to a common capacity C (max count rounded up to 8), and
core e runs a dense FFN for expert e on its token group. All operands are
pre-transposed / pre-cast to fp16 on the host (same PE rate as bf16, 8x the
mantissa: rel-err ~6e-4 vs ~4.5e-3) so the device kernel is pure matmul +
silu*mul with no on-chip transposes:
  core e computes outT = w_d @ (silu(w_gT.T @ xT) * (w_uT.T @ xT))
with xT [D, C], producing outT [D, C] fp32; the host transposes back and
unpermutes.
"""

import numpy as np
import ml_dtypes

import concourse.bass as bass
import concourse.mybir as mybir
from concourse import bacc
from concourse.tile import TileContext
from concourse.bass import ts, ds
from concourse.bass_utils import run_bass_kernel_spmd
from contextlib import ExitStack

BF16 = ml_dtypes.bfloat16
F16 = np.float16  # operand dtype for matmuls: same PE rate as bf16, 8x mantissa

D = 1024      # d_model
F = 2048      # d_ff
F2 = 2 * F    # gate+up
E = 8         # experts == cores
KD = D // 128   # 8  k-tiles over d_model
KF = F // 128   # 16 k-tiles over d_ff
MD = D // 128   # 8  m-tiles over d_model (output)
NT = 512        # token chunk (one PSUM bank at fp32)

_nc_cache = {}


def build_nc(C, repeats=1, hw_loop=0):
    """Build the per-core dense-FFN Bass program for token capacity C.

    repeats>1 re-emits the whole compute body (unrolled); hw_loop>0 wraps the
    body in a hardware For_i loop. Both are timing aids: slope of time vs
    repetition count isolates true exec time from dispatch overhead."""
    nc = bacc.Bacc("TRN2", target_bir_lowering=False, debug=False, num_devices=E)
    dt = mybir.dt
    xT = nc.dram_tensor("xT", [D, C], dt.bfloat16, kind="ExternalInput")
    wgu = nc.dram_tensor("wguT", [D, F2], dt.bfloat16, kind="ExternalInput")
    wd = nc.dram_tensor("wdT", [F, D], dt.bfloat16, kind="ExternalInput")
    outT = nc.dram_tensor("outT", [D, C], dt.float32, kind="ExternalOutput")

    with TileContext(nc) as tc, ExitStack() as ctx:
        wpool = ctx.enter_context(tc.tile_pool(name="weights", bufs=1))
        wgu_sb = wpool.tile([128, KD, F2], dt.bfloat16, tag="wgu")
        nc.sync.dma_start(wgu_sb[:], wgu.rearrange("(k p) f -> p k f", p=128))
        wd_sb = wpool.tile([128, KF, D], dt.bfloat16, tag="wd")
        nc.sync.dma_start(wd_sb[:], wd.rearrange("(k p) f -> p k f", p=128))

        xpool = ctx.enter_context(tc.tile_pool(name="x", bufs=2))
        hpool = ctx.enter_context(tc.tile_pool(name="h", bufs=2))
        spool = ctx.enter_context(tc.tile_pool(name="silu", bufs=4))
        opool = ctx.enter_context(tc.tile_pool(name="o", bufs=4))
        pg = ctx.enter_context(tc.tile_pool(name="pg", bufs=2, space="PSUM"))
        pu = ctx.enter_context(tc.tile_pool(name="pu", bufs=2, space="PSUM"))
        po = ctx.enter_context(tc.tile_pool(name="po", bufs=2, space="PSUM"))

        xT_r = xT.rearrange("(k p) t -> p k t", p=128)
        outT_r = outT.rearrange("(m p) t -> p m t", p=128)

        def body():
            for n0 in [i for _ in range(repeats) for i in range(0, C, NT)]:
                nt = min(NT, C - n0)
                x_sb = xpool.tile([128, KD, NT], dt.bfloat16, tag="x")
                nc.sync.dma_start(x_sb[:, :, :nt], xT_r[:, :, n0:n0 + nt])
                h_sb = hpool.tile([128, KF, NT], dt.bfloat16, tag="h")
                for mp in range(KF):
                    psg = pg.tile([128, NT], dt.float32, tag="pg")
                    for k in range(KD):
                        nc.tensor.matmul(
                            psg[:, :nt], lhsT=wgu_sb[:, k, ts(mp, 128)],
                            rhs=x_sb[:, k, :nt], start=(k == 0), stop=(k == KD - 1))
                    psu = pu.tile([128, NT], dt.float32, tag="pu")
                    for k in range(KD):
                        nc.tensor.matmul(
                            psu[:, :nt], lhsT=wgu_sb[:, k, ds(F + mp * 128, 128)],
                            rhs=x_sb[:, k, :nt], start=(k == 0), stop=(k == KD - 1))
                    sil = spool.tile([128, NT], dt.bfloat16, tag="sil")
                    nc.scalar.activation(sil[:, :nt], psg[:, :nt],
                                         mybir.ActivationFunctionType.Silu)
                    nc.vector.tensor_mul(h_sb[:, mp, :nt], sil[:, :nt], psu[:, :nt])
                for m in range(MD):
                    pso = po.tile([128, NT], dt.float32, tag="po")
                    for k in range(KF):
                        nc.tensor.matmul(
                            pso[:, :nt], lhsT=wd_sb[:, k, ts(m, 128)],
                            rhs=h_sb[:, k, :nt], start=(k == 0), stop=(k == KF - 1))
                    o_sb = opool.tile([128, NT], dt.float32, tag="o")
                    nc.vector.tensor_copy(o_sb[:, :nt], pso[:, :nt])
                    nc.sync.dma_start(outT_r[:, m, n0:n0 + nt], o_sb[:, :nt])

        if hw_loop:
            with tc.For_i(0, hw_loop, 1):
                body()
        else:
            body()
    nc.finalize()
    return nc


def build_nc_wide(C, hw_loop=0):
    """Variant: 1024-token compute chunks with [128,1024] PSUM tiles.

    - halves ACT/DVE eviction instruction count (wide silu/mul)
    - consecutive matmuls share the same lhsT (LDW dedup opportunity)
    - PSUM banks: pg 2x2 + pu 1x2 + po 2x1 = 8
    """
    nc = bacc.Bacc("TRN2", target_bir_lowering=False, debug=False, num_devices=E)
    dt = mybir.dt
    NW = 1024
    xT = nc.dram_tensor("xT", [D, C], dt.bfloat16, kind="ExternalInput")
    wgu = nc.dram_tensor("wguT", [D, F2], dt.bfloat16, kind="ExternalInput")
    wd = nc.dram_tensor("wdT", [F, D], dt.bfloat16, kind="ExternalInput")
    outT = nc.dram_tensor("outT", [D, C], dt.float32, kind="ExternalOutput")

    with TileContext(nc) as tc, ExitStack() as ctx:
        wpool = ctx.enter_context(tc.tile_pool(name="weights", bufs=1))
        wgu_sb = wpool.tile([128, KD, F2], dt.bfloat16, tag="wgu")
        nc.sync.dma_start(wgu_sb[:], wgu.rearrange("(k p) f -> p k f", p=128))
        wd_sb = wpool.tile([128, KF, D], dt.bfloat16, tag="wd")
        nc.sync.dma_start(wd_sb[:], wd.rearrange("(k p) f -> p k f", p=128))

        xpool = ctx.enter_context(tc.tile_pool(name="x", bufs=1))
        hpool = ctx.enter_context(tc.tile_pool(name="h", bufs=3))
        spool = ctx.enter_context(tc.tile_pool(name="silu", bufs=3))
        opool = ctx.enter_context(tc.tile_pool(name="o", bufs=4))
        pg = ctx.enter_context(tc.tile_pool(name="pg", bufs=2, space="PSUM"))
        pu = ctx.enter_context(tc.tile_pool(name="pu", bufs=1, space="PSUM"))
        po = ctx.enter_context(tc.tile_pool(name="po", bufs=2, space="PSUM"))

        xT_r = xT.rearrange("(k p) t -> p k t", p=128)
        outT_r = outT.rearrange("(m p) t -> p m t", p=128)

        def do_chunk(n0, nw):
            # nw tokens starting at n0; nw in {1024, C % 1024}
            nh = (nw + NT - 1) // NT  # h sub-chunks of <=512
            x_sb = xpool.tile([128, KD, NW], dt.bfloat16, tag="x")
            nc.sync.dma_start(x_sb[:, :, :nw], xT_r[:, :, n0:n0 + nw])
            h_sbs = [hpool.tile([128, KF, NT], dt.bfloat16, tag="h",
                                name=f"h_{n0}_{s}")
                     for s in range(nh)]
            for mp in range(KF):
                psg = pg.tile([128, NW], dt.float32, tag="pg")
                for k in range(KD):
                    for s in range(nh):
                        w = min(NT, nw - s * NT)
                        nc.tensor.matmul(
                            psg[:, s * NT:s * NT + w],
                            lhsT=wgu_sb[:, k, ts(mp, 128)],
                            rhs=x_sb[:, k, s * NT:s * NT + w],
                            start=(k == 0), stop=(k == KD - 1))
                psu = pu.tile([128, NW], dt.float32, tag="pu")
                for k in range(KD):
                    for s in range(nh):
                        w = min(NT, nw - s * NT)
                        nc.tensor.matmul(
                            psu[:, s * NT:s * NT + w],
                            lhsT=wgu_sb[:, k, ds(F + mp * 128, 128)],
                            rhs=x_sb[:, k, s * NT:s * NT + w],
                            start=(k == 0), stop=(k == KD - 1))
                sil = spool.tile([128, NW], dt.bfloat16, tag="sil")
                nc.scalar.activation(sil[:, :nw], psg[:, :nw],
                                     mybir.ActivationFunctionType.Silu)
                for s in range(nh):
                    w = min(NT, nw - s * NT)
                    nc.vector.tensor_mul(h_sbs[s][:, mp, :w],
                                         sil[:, s * NT:s * NT + w],
                                         psu[:, s * NT:s * NT + w])
            for m in range(MD):
                for s in range(nh):
                    w = min(NT, nw - s * NT)
                    pso = po.tile([128, NT], dt.float32, tag="po")
                    for k in range(KF):
                        nc.tensor.matmul(
                            pso[:, :w], lhsT=wd_sb[:, k, ts(m, 128)],
                            rhs=h_sbs[s][:, k, :w],
                            start=(k == 0), stop=(k == KF - 1))
                    o_sb = opool.tile([128, NT], dt.float32, tag="o")
                    nc.vector.tensor_copy(o_sb[:, :w], pso[:, :w])
                    nc.sync.dma_start(outT_r[:, m, n0 + s * NT:n0 + s * NT + w],
                                      o_sb[:, :w])

        def body():
            for n0 in range(0, C, NW):
                do_chunk(n0, min(NW, C - n0))

        if hw_loop:
            with tc.For_i(0, hw_loop, 1):
                body()
        else:
            body()
    nc.finalize()
    return nc


def get_nc(C):
    # build_nc_v7: TimelineSim 355.9us vs build_nc_big's 365.3us; fewer DMA
    # descriptors (~26 vs 184; each costs ~650ns of serial HWDGE issue) and
    # 1:3 LDW:MM in the down phase (vs 1:1).  HW loop-differential measures
    # ~433-447us/rep — at the machine's sustained matmul-stream limit: a
    # pure-MM microbench with zero DMA/deps measures 426us for the same
    # column count (PE effectively ~1.9GHz under sustained load, not 2.4).
    # fp8 DoubleRow (1.44x PE) was evaluated and rejected: e4m3 operands
    # give rel-err ~7e-2 on this problem vs the 2e-2 gate (measured in
    # numpy emulation; error is mantissa-limited, scaling cannot fix it).
    if C not in _nc_cache:
        _nc_cache[C] = build_nc_v7(C)
    return _nc_cache[C]


def build_nc_big(C, hw_loop=0):
    """Variant: 1536-token chunks ([128,1536] 3-bank PSUM tiles).

    Streams 3x512 tokens per weight load (LDW count 1920 -> ~768), evicts
    gate via silu into a chunk-resident SBUF tensor, then multiplies the up
    projection into it in place. PSUM: pp 2x3 + po 2x1 = 8 banks.
    """
    nc = bacc.Bacc("TRN2", target_bir_lowering=False, debug=False, num_devices=E)
    dt = mybir.dt
    NB = 1536
    xT = nc.dram_tensor("xT", [D, C], dt.float16, kind="ExternalInput")
    wgu = nc.dram_tensor("wguT", [D, F2], dt.float16, kind="ExternalInput")
    wd = nc.dram_tensor("wdT", [F, D], dt.float16, kind="ExternalInput")
    outT = nc.dram_tensor("outT", [D, C], dt.float32, kind="ExternalOutput")

    with TileContext(nc) as tc, ExitStack() as ctx:
        # per-k weight tiles with separate DMAs; the first chunk's x tiles
        # are DMA'd BEFORE the weights (see do_chunk) so the PE's first
        # matmul group is gated on ~4 MB, not the full 16 MB input set.
        wpool = ctx.enter_context(tc.tile_pool(name="weights", bufs=1))
        wgu_k = [wpool.tile([128, F2], dt.float16, tag=f"wgu{k}",
                            name=f"wgu{k}") for k in range(KD)]
        wd_k = [wpool.tile([128, D], dt.float16, tag=f"wd{k}",
                           name=f"wd{k}") for k in range(KF)]

        xpool = ctx.enter_context(tc.tile_pool(name="x", bufs=1))
        ghpool = ctx.enter_context(tc.tile_pool(name="gh", bufs=1))
        opool = ctx.enter_context(tc.tile_pool(name="o", bufs=4))
        pp = ctx.enter_context(tc.tile_pool(name="pp", bufs=2, space="PSUM"))
        po = ctx.enter_context(tc.tile_pool(name="po", bufs=2, space="PSUM"))

        xT_r = xT.rearrange("(k p) t -> p k t", p=128)
        outT_r = outT.rearrange("(m p) t -> p m t", p=128)

        def slices(nw):
            return [(s, min(NT, nw - s)) for s in range(0, nw, NT)]

        def do_chunk(n0, nw):
            x_sb = xpool.tile([128, KD, NB], dt.float16, tag="x")
            if first[0]:
                first[0] = False
                # interleave x and gate/up weight k-tiles so the first
                # matmul group's operands stream in consumption order
                for k in range(KD):
                    nc.sync.dma_start(x_sb[:, k, :nw], xT_r[:, k, n0:n0 + nw])
                    # first 512 f-columns land first so the k-th LDW of the
                    # first gate group unblocks after ~0.4 MB, not 1 MB
                    nc.sync.dma_start(wgu_k[k][:, :NT],
                                      wgu[k * 128:(k + 1) * 128, :NT])
                    nc.sync.dma_start(wgu_k[k][:, NT:],
                                      wgu[k * 128:(k + 1) * 128, NT:])
                for k in range(KF):
                    nc.sync.dma_start(wd_k[k][:], wd[k * 128:(k + 1) * 128, :])
            else:
                for k in range(KD):
                    nc.sync.dma_start(x_sb[:, k, :nw], xT_r[:, k, n0:n0 + nw])
            gh = ghpool.tile([128, KF, NB], dt.float16, tag="gh")
            for phase in (0, 1):  # 0: gate+silu, 1: up+mul-in-place
                for mp in range(KF):
                    ps = pp.tile([128, NB], dt.float32, tag="pp",
                                 name=f"ps_{n0}_{phase}_{mp}")
                    f0 = mp * 128 if phase == 0 else F + mp * 128
                    for k in range(KD):
                        for s, w in slices(nw):
                            nc.tensor.matmul(
                                ps[:, s:s + w],
                                lhsT=wgu_k[k][:, ds(f0, 128)],
                                rhs=x_sb[:, k, s:s + w],
                                start=(k == 0), stop=(k == KD - 1))
                    if phase == 0:
                        nc.scalar.activation(gh[:, mp, :nw], ps[:, :nw],
                                             mybir.ActivationFunctionType.Silu)
                    else:
                        nc.vector.tensor_mul(gh[:, mp, :nw], gh[:, mp, :nw],
                                             ps[:, :nw])
            for m in range(MD):
                for s, w in slices(nw):
                    pso = po.tile([128, NT], dt.float32, tag="po",
                                  name=f"pso_{n0}_{m}_{s}")
                    for k in range(KF):
                        nc.tensor.matmul(
                            pso[:, :w], lhsT=wd_k[k][:, ts(m, 128)],
                            rhs=gh[:, k, s:s + w],
                            start=(k == 0), stop=(k == KF - 1))
                    o_sb = opool.tile([128, NT], dt.float32, tag="o",
                                      name=f"o_{n0}_{m}_{s}")
                    nc.vector.tensor_copy(o_sb[:, :w], pso[:, :w])
                    nc.sync.dma_start(outT_r[:, m, n0 + s:n0 + s + w],
                                      o_sb[:, :w])

        first = [True]

        def body():
            # smallest chunk first: the cold-start stall is gated on the
            # first chunk's x DMA, so lead with the cheapest one
            chunks = [(n0, min(NB, C - n0)) for n0 in range(0, C, NB)]
            chunks.sort(key=lambda c: c[1])
            for n0, nw in chunks:
                do_chunk(n0, nw)



        if hw_loop:
            with tc.For_i(0, hw_loop, 1):
                body()
        else:
            body()
    nc.finalize()
    return nc


def build_nc_v3(C, hw_loop=0):
    """Tuned variant of build_nc_big:

    - weight DMAs striped in consumption order (512-col stripes across all
      k-tiles, gate half first, then up half, then wd) so the first gate
      phase is never DMA-starved;
    - remainder-chunk token slices equalized (e.g. 568 -> 284+284, not
      512+56) so no runt matmuls pay the per-MM issue floor;
    - down phase restructured m -> k -> s with a full-width [128, NB] PSUM
      tile from the shared pool: 1 LDW per 3 matmuls instead of 1:1.
    PSUM: ps pool 2x3 banks = 6 of 8 banks.
    """
    nc = bacc.Bacc("TRN2", target_bir_lowering=False, debug=False, num_devices=E)
    dt = mybir.dt
    NB = 1536
    xT = nc.dram_tensor("xT", [D, C], dt.float16, kind="ExternalInput")
    wgu = nc.dram_tensor("wguT", [D, F2], dt.float16, kind="ExternalInput")
    wd = nc.dram_tensor("wdT", [F, D], dt.float16, kind="ExternalInput")
    outT = nc.dram_tensor("outT", [D, C], dt.float32, kind="ExternalOutput")

    with TileContext(nc) as tc, ExitStack() as ctx:
        wpool = ctx.enter_context(tc.tile_pool(name="weights", bufs=1))
        wgu_k = [wpool.tile([128, F2], dt.float16, tag=f"wgu{k}",
                            name=f"wgu{k}") for k in range(KD)]
        wd_k = [wpool.tile([128, D], dt.float16, tag=f"wd{k}",
                           name=f"wd{k}") for k in range(KF)]

        xpool = ctx.enter_context(tc.tile_pool(name="x", bufs=1))
        ghpool = ctx.enter_context(tc.tile_pool(name="gh", bufs=1))
        opool = ctx.enter_context(tc.tile_pool(name="o", bufs=2))
        pspool = ctx.enter_context(tc.tile_pool(name="ps", bufs=2, space="PSUM"))

        xT_r = xT.rearrange("(k p) t -> p k t", p=128)
        outT_r = outT.rearrange("(m p) t -> p m t", p=128)

        def slices(nw):
            ns = (nw + NT - 1) // NT
            w = -(-nw // ns)  # equal widths, last may be smaller by <ns
            return [(s, min(w, nw - s)) for s in range(0, nw, w)]

        def chunk_list():
            chunks = []
            rem = C
            while rem > 0:
                take = NB if rem >= NB else rem
                chunks.append(take)
                rem -= take
            sizes = sorted(chunks)  # smallest first: cheapest cold start
            offs = []
            n0 = 0
            for s in sizes:
                offs.append((n0, s))
                n0 += s
            return offs

        def emit_weight_dmas():
            # gate half, then up half: 512-col stripes across all k-tiles in
            # the order the first gate phase consumes them
            for half in (0, F):
                for c0 in range(half, half + F, 512):
                    for k in range(KD):
                        nc.sync.dma_start(wgu_k[k][:, c0:c0 + 512],
                                          wgu[k * 128:(k + 1) * 128, c0:c0 + 512])
            for c0 in range(0, D, 512):
                for k in range(KF):
                    nc.sync.dma_start(wd_k[k][:, c0:c0 + 512],
                                      wd[k * 128:(k + 1) * 128, c0:c0 + 512])

        def do_chunk(n0, nw, first_chunk):
            x_sb = xpool.tile([128, KD, NB], dt.float16, tag="x")
            if first_chunk:
                for k in range(KD):
                    nc.sync.dma_start(x_sb[:, k, :nw], xT_r[:, k, n0:n0 + nw])
                emit_weight_dmas()
            else:
                for k in range(KD):
                    nc.sync.dma_start(x_sb[:, k, :nw], xT_r[:, k, n0:n0 + nw])
            gh = ghpool.tile([128, KF, NB], dt.float16, tag="gh")
            for phase in (0, 1):  # 0: gate+silu, 1: up+mul-in-place
                for mp in range(KF):
                    ps = pspool.tile([128, NB], dt.float32, tag="ps",
                                     name=f"ps_{n0}_{phase}_{mp}")
                    f0 = mp * 128 if phase == 0 else F + mp * 128
                    for k in range(KD):
                        for s, w in slices(nw):
                            nc.tensor.matmul(
                                ps[:, s:s + w],
                                lhsT=wgu_k[k][:, ds(f0, 128)],
                                rhs=x_sb[:, k, s:s + w],
                                start=(k == 0), stop=(k == KD - 1))
                    if phase == 0:
                        nc.scalar.activation(gh[:, mp, :nw], ps[:, :nw],
                                             mybir.ActivationFunctionType.Silu)
                    else:
                        nc.vector.tensor_mul(gh[:, mp, :nw], gh[:, mp, :nw],
                                             ps[:, :nw])
            for m in range(MD):
                pso = pspool.tile([128, NB], dt.float32, tag="ps",
                                  name=f"pso_{n0}_{m}")
                for k in range(KF):
                    for s, w in slices(nw):
                        nc.tensor.matmul(
                            pso[:, s:s + w], lhsT=wd_k[k][:, ts(m, 128)],
                            rhs=gh[:, k, s:s + w],
                            start=(k == 0), stop=(k == KF - 1))
                o_sb = opool.tile([128, NB], dt.float32, tag="o",
                                  name=f"o_{n0}_{m}")
                nc.vector.tensor_copy(o_sb[:, :nw], pso[:, :nw])
                nc.sync.dma_start(outT_r[:, m, n0:n0 + nw], o_sb[:, :nw])

        def body():
            for i, (n0, nw) in enumerate(chunk_list()):
                do_chunk(n0, nw, i == 0)

        if hw_loop:
            with tc.For_i(0, hw_loop, 1):
                body()
        else:
            body()
    nc.finalize()
    return nc


def build_nc_v4(C, hw_loop=0):
    """v3 + better chunking and cold-start:

    - chunks [first(1 slice), 1536*q (3x512 slices), last(1 slice)]: the
      first/last remainder chunks absorb C%512 as two ~equal >=256-token
      1-slice chunks, so every matmul is >=256 wide (no runt-MM issue-floor
      waste), every PSUM write is bank-aligned, and both the cold-start x
      DMA and the serial eviction tail after the last matmul are small;
    - the first 512 gate-weight columns stream in 128-col pieces so the
      first matmul group unblocks after ~0.3 MB.
    """
    nc = bacc.Bacc("TRN2", target_bir_lowering=False, debug=False, num_devices=E)
    dt = mybir.dt
    NB = 1536
    xT = nc.dram_tensor("xT", [D, C], dt.float16, kind="ExternalInput")
    wgu = nc.dram_tensor("wguT", [D, F2], dt.float16, kind="ExternalInput")
    wd = nc.dram_tensor("wdT", [F, D], dt.float16, kind="ExternalInput")
    outT = nc.dram_tensor("outT", [D, C], dt.float32, kind="ExternalOutput")

    with TileContext(nc) as tc, ExitStack() as ctx:
        wpool = ctx.enter_context(tc.tile_pool(name="weights", bufs=1))
        wgu_k = [wpool.tile([128, F2], dt.float16, tag=f"wgu{k}",
                            name=f"wgu{k}") for k in range(KD)]
        wd_k = [wpool.tile([128, D], dt.float16, tag=f"wd{k}",
                           name=f"wd{k}") for k in range(KF)]

        xpool = ctx.enter_context(tc.tile_pool(name="x", bufs=1))
        ghpool = ctx.enter_context(tc.tile_pool(name="gh", bufs=1))
        opool = ctx.enter_context(tc.tile_pool(name="o", bufs=2))
        pspool = ctx.enter_context(tc.tile_pool(name="ps", bufs=2, space="PSUM"))

        xT_r = xT.rearrange("(k p) t -> p k t", p=128)
        outT_r = outT.rearrange("(m p) t -> p m t", p=128)

        def chunk_list():
            """[(n0, nw, [slice widths])]; each chunk is 1 slice of any
            width, or all-512 slices (bank alignment for free)."""
            ns = -(-C // NT)
            if ns <= 3:
                w = -(-C // ns)
                widths = [min(w, C - i * w) for i in range(ns)]
                groups = [[wi] for wi in widths]
            else:
                slack = NT * ns - C
                wf = NT - (slack + 1) // 2
                wl = NT - slack // 2
                mid = [NT] * (ns - 2)
                groups = [[wf]] + [mid[i:i + 3] for i in range(0, len(mid), 3)] \
                    + [[wl]]
            out, n0 = [], 0
            for g in groups:
                out.append((n0, sum(g), g))
                n0 += sum(g)
            return out

        def emit_weight_dmas():
            for c0 in range(0, 512, 128):  # first gate stripe: fine-grained
                for k in range(KD):
                    nc.sync.dma_start(wgu_k[k][:, c0:c0 + 128],
                                      wgu[k * 128:(k + 1) * 128, c0:c0 + 128])
            for half in (0, F):
                for c0 in range(half, half + F, 512):
                    if c0 == 0:
                        continue  # already emitted fine-grained
                    for k in range(KD):
                        nc.sync.dma_start(wgu_k[k][:, c0:c0 + 512],
                                          wgu[k * 128:(k + 1) * 128, c0:c0 + 512])
            for c0 in range(0, D, 512):
                for k in range(KF):
                    nc.sync.dma_start(wd_k[k][:, c0:c0 + 512],
                                      wd[k * 128:(k + 1) * 128, c0:c0 + 512])

        def do_chunk(n0, nw, widths, first_chunk):
            x_sb = xpool.tile([128, KD, NB], dt.float16, tag="x")
            for k in range(KD):
                nc.sync.dma_start(x_sb[:, k, :nw], xT_r[:, k, n0:n0 + nw])
            if first_chunk:
                emit_weight_dmas()
            slc = []
            s = 0
            for w in widths:
                slc.append((s, w))
                s += w
            gh = ghpool.tile([128, KF, NB], dt.float16, tag="gh")
            for phase in (0, 1):  # 0: gate+silu, 1: up+mul-in-place
                for mp in range(KF):
                    ps = pspool.tile([128, NB], dt.float32, tag="ps",
                                     name=f"ps_{n0}_{phase}_{mp}")
                    f0 = mp * 128 if phase == 0 else F + mp * 128
                    for k in range(KD):
                        for s, w in slc:
                            nc.tensor.matmul(
                                ps[:, s:s + w],
                                lhsT=wgu_k[k][:, ds(f0, 128)],
                                rhs=x_sb[:, k, s:s + w],
                                start=(k == 0), stop=(k == KD - 1))
                    if phase == 0:
                        nc.scalar.activation(gh[:, mp, :nw], ps[:, :nw],
                                             mybir.ActivationFunctionType.Silu)
                    else:
                        nc.vector.tensor_mul(gh[:, mp, :nw], gh[:, mp, :nw],
                                             ps[:, :nw])
            for m in range(MD):
                pso = pspool.tile([128, NB], dt.float32, tag="ps",
                                  name=f"pso_{n0}_{m}")
                for k in range(KF):
                    for s, w in slc:
                        nc.tensor.matmul(
                            pso[:, s:s + w], lhsT=wd_k[k][:, ts(m, 128)],
                            rhs=gh[:, k, s:s + w],
                            start=(k == 0), stop=(k == KF - 1))
                o_sb = opool.tile([128, NB], dt.float32, tag="o",
                                  name=f"o_{n0}_{m}")
                nc.vector.tensor_copy(o_sb[:, :nw], pso[:, :nw])
                nc.sync.dma_start(outT_r[:, m, n0:n0 + nw], o_sb[:, :nw])

        def body():
            for i, (n0, nw, widths) in enumerate(chunk_list()):
                do_chunk(n0, nw, widths, i == 0)

        if hw_loop:
            with tc.For_i(0, hw_loop, 1):
                body()
        else:
            body()
    nc.finalize()
    return nc


def build_nc_v5(C, hw_loop=0):
    """Bank-safe tuned variant (the successor of build_nc_big):

    - chunks [rem (2 equal slices), 1536 (3x512)...]: every matmul is
      >=256 tokens wide and every PSUM write sits in its own bank-aligned
      512-column slot (slice i of a chunk lives at psum column 512*i);
    - silu/mul/eviction/out-DMA run per-slice, so the serial tail after the
      very last matmul is one 512-wide eviction, not a whole 1536 chunk;
    - weight DMAs stream in consumption order (gate stripes across k first,
      the first stripe in 128-col pieces, then up half, then down weights);
    - down phase is m -> k -> slice with a full-width PSUM tile: one
      weight load per 3 matmuls.
    PSUM: shared ps pool 2x3 banks = 6 of 8 banks.
    """
    nc = bacc.Bacc("TRN2", target_bir_lowering=False, debug=False, num_devices=E)
    dt = mybir.dt
    NB = 1536
    xT = nc.dram_tensor("xT", [D, C], dt.float16, kind="ExternalInput")
    wgu = nc.dram_tensor("wguT", [D, F2], dt.float16, kind="ExternalInput")
    wd = nc.dram_tensor("wdT", [F, D], dt.float16, kind="ExternalInput")
    outT = nc.dram_tensor("outT", [D, C], dt.float32, kind="ExternalOutput")

    with TileContext(nc) as tc, ExitStack() as ctx:
        wpool = ctx.enter_context(tc.tile_pool(name="weights", bufs=1))
        wgu_k = [wpool.tile([128, F2], dt.float16, tag=f"wgu{k}",
                            name=f"wgu{k}") for k in range(KD)]
        wd_k = [wpool.tile([128, D], dt.float16, tag=f"wd{k}",
                           name=f"wd{k}") for k in range(KF)]

        xpool = ctx.enter_context(tc.tile_pool(name="x", bufs=1))
        ghpool = ctx.enter_context(tc.tile_pool(name="gh", bufs=1))
        opool = ctx.enter_context(tc.tile_pool(name="o", bufs=2))
        pspool = ctx.enter_context(tc.tile_pool(name="ps", bufs=2, space="PSUM"))

        xT_r = xT.rearrange("(k p) t -> p k t", p=128)
        outT_r = outT.rearrange("(m p) t -> p m t", p=128)

        def chunk_list():
            """[(n0, nw, [(tok_off, psum_off, w), ...])] — remainder first
            (2 equal slices), then full 1536 chunks (3x512)."""
            rem = C % NB
            chunks = []
            if rem:
                if rem <= NT:
                    w0 = (rem + 1) // 2
                    widths = [w0, rem - w0] if rem - w0 else [w0]
                elif rem <= 2 * NT:
                    w0 = (rem + 1) // 2
                    widths = [w0, rem - w0]
                else:
                    w0 = (rem + 2) // 3
                    widths = [w0, w0, rem - 2 * w0]
                chunks.append(widths)
            chunks += [[NT, NT, NT]] * (C // NB)
            out, n0 = [], 0
            for widths in chunks:
                slc, t = [], 0
                for i, w in enumerate(widths):
                    slc.append((t, i * NT, w))
                    t += w
                out.append((n0, sum(widths), slc))
                n0 += sum(widths)
            return out

        def emit_weight_dmas():
            for c0 in range(0, 512, 128):  # first gate stripe: fine-grained
                for k in range(KD):
                    nc.sync.dma_start(wgu_k[k][:, c0:c0 + 128],
                                      wgu[k * 128:(k + 1) * 128, c0:c0 + 128])
            for half in (0, F):
                for c0 in range(half, half + F, 512):
                    if c0 == 0:
                        continue  # emitted fine-grained above
                    for k in range(KD):
                        nc.sync.dma_start(wgu_k[k][:, c0:c0 + 512],
                                          wgu[k * 128:(k + 1) * 128, c0:c0 + 512])
            for c0 in range(0, D, 512):
                for k in range(KF):
                    nc.sync.dma_start(wd_k[k][:, c0:c0 + 512],
                                      wd[k * 128:(k + 1) * 128, c0:c0 + 512])

        def do_chunk(n0, nw, slc, first_chunk):
            x_sb = xpool.tile([128, KD, NB], dt.float16, tag="x")
            for k in range(KD):
                nc.sync.dma_start(x_sb[:, k, :nw], xT_r[:, k, n0:n0 + nw])
            if first_chunk:
                emit_weight_dmas()
            gh = ghpool.tile([128, KF, NB], dt.float16, tag="gh")
            for phase in (0, 1):  # 0: gate+silu, 1: up+mul-in-place
                for mp in range(KF):
                    ps = pspool.tile([128, NB], dt.float32, tag="ps",
                                     name=f"ps_{n0}_{phase}_{mp}")
                    f0 = mp * 128 if phase == 0 else F + mp * 128
                    for k in range(KD):
                        for t, p, w in slc:
                            nc.tensor.matmul(
                                ps[:, p:p + w],
                                lhsT=wgu_k[k][:, ds(f0, 128)],
                                rhs=x_sb[:, k, t:t + w],
                                start=(k == 0), stop=(k == KD - 1))
                    for t, p, w in slc:
                        if phase == 0:
                            nc.scalar.activation(
                                gh[:, mp, t:t + w], ps[:, p:p + w],
                                mybir.ActivationFunctionType.Silu)
                        else:
                            nc.vector.tensor_mul(gh[:, mp, t:t + w],
                                                 gh[:, mp, t:t + w],
                                                 ps[:, p:p + w])
            for m in range(MD):
                pso = pspool.tile([128, NB], dt.float32, tag="ps",
                                  name=f"pso_{n0}_{m}")
                for k in range(KF):
                    for t, p, w in slc:
                        nc.tensor.matmul(
                            pso[:, p:p + w], lhsT=wd_k[k][:, ts(m, 128)],
                            rhs=gh[:, k, t:t + w],
                            start=(k == 0), stop=(k == KF - 1))
                o_sb = opool.tile([128, NB], dt.float32, tag="o",
                                  name=f"o_{n0}_{m}")
                for t, p, w in slc:
                    nc.vector.tensor_copy(o_sb[:, t:t + w], pso[:, p:p + w])
                    nc.sync.dma_start(outT_r[:, m, n0 + t:n0 + t + w],
                                      o_sb[:, t:t + w])

        def body():
            for i, (n0, nw, slc) in enumerate(chunk_list()):
                do_chunk(n0, nw, slc, i == 0)

        if hw_loop:
            with tc.For_i(0, hw_loop, 1):
                body()
        else:
            body()
    nc.finalize()
    return nc


def build_nc_v6(C, hw_loop=0):
    """Uniform 2-slice chunks + few big DMAs + 3-deep PSUM pool.

    TimelineSim showed two costs the 1536-chunk builds pay: (a) each
    dma_start costs ~650ns of serial issue on the sync queue, so per-k /
    per-stripe descriptor spam delays the first matmul by ~8us; (b) with
    2x3-bank PSUM tiles the silu/mul round-trip doesn't fit the 2-buffer
    recycle window for narrow slices, stalling PE ~0.4us per group.

    Here: chunks are ceil(C/1024) near-equal sizes, each 2 bank-aligned
    slices -> PSUM tiles are [128,1024] (2 banks) and the pool holds 3
    bufs (6 banks): two full groups of recycle slack. Weights live in two
    monolithic SBUF tiles so each 512-col stripe (all k-tiles) is ONE
    descriptor, ordered gate-half, up-half, down; x streams one descriptor
    per slice.
    """
    nc = bacc.Bacc("TRN2", target_bir_lowering=False, debug=False, num_devices=E)
    dt = mybir.dt
    NBC = 1024
    xT = nc.dram_tensor("xT", [D, C], dt.float16, kind="ExternalInput")
    wgu = nc.dram_tensor("wguT", [D, F2], dt.float16, kind="ExternalInput")
    wd = nc.dram_tensor("wdT", [F, D], dt.float16, kind="ExternalInput")
    outT = nc.dram_tensor("outT", [D, C], dt.float32, kind="ExternalOutput")

    with TileContext(nc) as tc, ExitStack() as ctx:
        wpool = ctx.enter_context(tc.tile_pool(name="weights", bufs=1))
        wgu_sb = wpool.tile([128, KD, F2], dt.float16, tag="wgu")
        wd_sb = wpool.tile([128, KF, D], dt.float16, tag="wd")

        xpool = ctx.enter_context(tc.tile_pool(name="x", bufs=2))
        ghpool = ctx.enter_context(tc.tile_pool(name="gh", bufs=1))
        opool = ctx.enter_context(tc.tile_pool(name="o", bufs=3))
        pspool = ctx.enter_context(tc.tile_pool(name="ps", bufs=3, space="PSUM"))

        xT_r = xT.rearrange("(k p) t -> p k t", p=128)
        wgu_r = wgu.rearrange("(k p) f -> p k f", p=128)
        wd_r = wd.rearrange("(k p) m -> p k m", p=128)
        outT_r = outT.rearrange("(m p) t -> p m t", p=128)

        def chunk_list():
            """[(n0, nw, [(tok_off, psum_off, w), ...])] near-equal 2-slice
            chunks."""
            nch = -(-C // NBC)
            base, ext = divmod(C, nch)
            sizes = [base + (1 if i < ext else 0) for i in range(nch)]
            out, n0 = [], 0
            for nw in sizes:
                w0 = (nw + 1) // 2
                slc = [(0, 0, w0)]
                if nw - w0:
                    slc.append((w0, NT, nw - w0))
                out.append((n0, nw, slc))
                n0 += nw
            return out

        def emit_weight_dmas():
            # gate half first, leading 512 cols in two 256-col pieces so the
            # first matmul group unblocks after ~0.5 MB
            for c0, c1 in [(0, 256), (256, 512)] + [
                    (c, c + 512) for c in range(512, F, 512)]:
                nc.sync.dma_start(wgu_sb[:, :, c0:c1], wgu_r[:, :, c0:c1])
            for c0 in range(F, F2, 512):
                nc.sync.dma_start(wgu_sb[:, :, c0:c0 + 512],
                                  wgu_r[:, :, c0:c0 + 512])
            for c0 in range(0, D, 512):
                nc.sync.dma_start(wd_sb[:, :, c0:c0 + 512],
                                  wd_r[:, :, c0:c0 + 512])

        def do_chunk(n0, nw, slc, first_chunk):
            x_sb = xpool.tile([128, KD, NBC], dt.float16, tag="x")
            for t, p, w in slc:
                nc.sync.dma_start(x_sb[:, :, t:t + w], xT_r[:, :, n0 + t:n0 + t + w])
                if first_chunk and t == 0:
                    emit_weight_dmas()
            gh = ghpool.tile([128, KF, NBC], dt.float16, tag="gh")
            for phase in (0, 1):  # 0: gate+silu, 1: up+mul-in-place
                for mp in range(KF):
                    ps = pspool.tile([128, NBC], dt.float32, tag="ps",
                                     name=f"ps_{n0}_{phase}_{mp}")
                    f0 = mp * 128 if phase == 0 else F + mp * 128
                    for k in range(KD):
                        for t, p, w in slc:
                            nc.tensor.matmul(
                                ps[:, p:p + w],
                                lhsT=wgu_sb[:, k, ds(f0, 128)],
                                rhs=x_sb[:, k, t:t + w],
                                start=(k == 0), stop=(k == KD - 1))
                    for t, p, w in slc:
                        if phase == 0:
                            nc.scalar.activation(
                                gh[:, mp, t:t + w], ps[:, p:p + w],
                                mybir.ActivationFunctionType.Silu)
                        else:
                            nc.vector.tensor_mul(gh[:, mp, t:t + w],
                                                 gh[:, mp, t:t + w],
                                                 ps[:, p:p + w])
            for m in range(MD):
                pso = pspool.tile([128, NBC], dt.float32, tag="ps",
                                  name=f"pso_{n0}_{m}")
                for k in range(KF):
                    for t, p, w in slc:
                        nc.tensor.matmul(
                            pso[:, p:p + w], lhsT=wd_sb[:, k, ts(m, 128)],
                            rhs=gh[:, k, t:t + w],
                            start=(k == 0), stop=(k == KF - 1))
                o_sb = opool.tile([128, NBC], dt.float32, tag="o",
                                  name=f"o_{n0}_{m}")
                for t, p, w in slc:
                    nc.vector.tensor_copy(o_sb[:, t:t + w], pso[:, p:p + w])
                    nc.sync.dma_start(outT_r[:, m, n0 + t:n0 + t + w],
                                      o_sb[:, t:t + w])

        def body():
            for i, (n0, nw, slc) in enumerate(chunk_list()):
                do_chunk(n0, nw, slc, i == 0)

        if hw_loop:
            with tc.For_i(0, hw_loop, 1):
                body()
        else:
            body()
    nc.finalize()
    return nc


def build_nc_mmonly(C, hw_loop=0):
    """Microbench: gate-phase-like pure matmul stream (resident operands).
    Per-rep predicted 2.4GHz time: C*128/2.4e9 ns. Measures real PE rate."""
    nc = bacc.Bacc("TRN2", target_bir_lowering=False, debug=False, num_devices=E)
    dt = mybir.dt
    xT = nc.dram_tensor("xT", [D, C], dt.float16, kind="ExternalInput")
    wgu = nc.dram_tensor("wguT", [D, F2], dt.float16, kind="ExternalInput")
    outT = nc.dram_tensor("outT", [D, C], dt.float32, kind="ExternalOutput")
    NBC = 512
    with TileContext(nc) as tc, ExitStack() as ctx:
        wpool = ctx.enter_context(tc.tile_pool(name="weights", bufs=1))
        wg_sb = wpool.tile([128, KD, F2], dt.float16, tag="wg")
        xpool = ctx.enter_context(tc.tile_pool(name="x", bufs=1))
        x_sb = xpool.tile([128, KD, NBC], dt.float16, tag="x")
        gpool = ctx.enter_context(tc.tile_pool(name="g", bufs=2))
        pspool = ctx.enter_context(tc.tile_pool(name="ps", bufs=4, space="PSUM"))
        nc.sync.dma_start(x_sb[:], xT.rearrange("(k p) t -> p k t", p=128)[:, :, :NBC])
        for k in range(KD):
            nc.sync.dma_start(wg_sb[:, k], wgu.rearrange("(k p) f -> p k f", p=128)[:, k])

        def body():
            # same MM count as one full v7 rep-worth of gate+up+down per
            # 512 tokens x (C/512): 384 * ceil(C/512) MMs of N=512
            for rep in range(-(-C // NBC)):
                for mp in range(KF * 2 + MD):
                    ps = pspool.tile([128, NBC], dt.float32, tag="ps",
                                     name=f"ps_{rep}_{mp}")
                    f0 = (mp * 128) % F2
                    for k in range(KD):
                        nc.tensor.matmul(
                            ps[:], lhsT=wg_sb[:, k, ds(f0, 128)],
                            rhs=x_sb[:, k, :],
                            start=(k == 0), stop=(k == KD - 1))
                    g_sb = gpool.tile([128, NBC], dt.float32, tag="g",
                                      name=f"g_{rep}_{mp}")
                    nc.scalar.activation(g_sb[:], ps[:],
                                         mybir.ActivationFunctionType.Silu)
            nc.sync.dma_start(
                outT.rearrange("(m p) t -> p m t", p=128)[:, 0, :NBC],
                g_sb[:])

        if hw_loop:
            with tc.For_i(0, hw_loop, 1):
                body()
        else:
            body()
    nc.finalize()
    return nc


def build_nc_v8(C, hw_loop=0):
    """v7 + weight streams spread across the first chunk's phases."""
    return build_nc_v7(C, hw_loop=hw_loop, spread_weights=True)


def build_nc_v7(C, hw_loop=0, weights_outside=False, spread_weights=False):
    """Near-equal 3-slice chunks + stripe-major weight tiles.

    Design notes (from TimelineSim analysis of big/v5/v6):
    - each dma_start costs ~650ns serial issue -> few, large descriptors;
    - Tile dep-tracking uses flattened-free-dim bounding boxes -> weight
      tiles are laid out stripe-major ([128, stripe, k, cols]) so one
      stripe DMA = one exact-bbox descriptor;
    - PSUM recycle (matmul group -> silu/mul -> free) takes ~2.3us, so
      chunk slices are sized so a group is >=3us: near-equal chunks of
      ~1052 tokens, 3 bank-aligned slices each, [128,1536] psum x2 bufs;
    - gate weights stream in 256-col stripes (consumption order), up half
      and down weights in 512-col stripes;
    - first chunk's first two gate groups run slice-outer so the first
      matmul needs only slice0 of x + the first gate stripe (~1 MB).
    """
    nc = bacc.Bacc("TRN2", target_bir_lowering=False, debug=False, num_devices=E)
    dt = mybir.dt
    NB = 1536
    xT = nc.dram_tensor("xT", [D, C], dt.float16, kind="ExternalInput")
    wgu = nc.dram_tensor("wguT", [D, F2], dt.float16, kind="ExternalInput")
    wd = nc.dram_tensor("wdT", [F, D], dt.float16, kind="ExternalInput")
    outT = nc.dram_tensor("outT", [D, C], dt.float32, kind="ExternalOutput")

    GS = 256   # gate-half weight stripe width
    WS = 512   # up-half / down weight stripe width
    NGS = F // GS
    with TileContext(nc) as tc, ExitStack() as ctx:
        wpool = ctx.enter_context(tc.tile_pool(name="weights", bufs=1))
        # [128, stripe, k, cols]: one DMA per stripe with an exact bbox
        wg_sb = wpool.tile([128, NGS, KD, GS], dt.float16, tag="wg")
        wu_sb = wpool.tile([128, F // WS, KD, WS], dt.float16, tag="wu")
        wd_sb = wpool.tile([128, D // WS, KF, WS], dt.float16, tag="wd")

        xpool = ctx.enter_context(tc.tile_pool(name="x", bufs=1))
        ghpool = ctx.enter_context(tc.tile_pool(name="gh", bufs=1))
        opool = ctx.enter_context(tc.tile_pool(name="o", bufs=2))
        pspool = ctx.enter_context(tc.tile_pool(name="ps", bufs=2, space="PSUM"))

        xT_r = xT.rearrange("(k p) t -> p k t", p=128)
        wgu_r = wgu.rearrange("(k p) f -> p k f", p=128)
        wd_r = wd.rearrange("(k p) m -> p k m", p=128)
        outT_r = outT.rearrange("(m p) t -> p m t", p=128)

        def gate_w(mp):  # lhsT for gate col-tile mp (128 cols)
            f0 = mp * 128
            return wg_sb[:, f0 // GS, :, (f0 % GS):(f0 % GS) + 128]

        def up_w(mp):
            f0 = mp * 128
            return wu_sb[:, f0 // WS, :, (f0 % WS):(f0 % WS) + 128]

        def down_w(m):
            f0 = m * 128
            return wd_sb[:, f0 // WS, :, (f0 % WS):(f0 % WS) + 128]

        def chunk_list():
            nch = max(1, -(-C // NB))
            base, ext = divmod(C, nch)
            sizes = [base + (1 if i < ext else 0) for i in range(nch)]
            out, n0 = [], 0
            for nw in sizes:
                ns = min(3, -(-nw // NT))
                wv, we = divmod(nw, ns)
                widths = [wv + (1 if i < we else 0) for i in range(ns)]
                slc, t = [], 0
                for i, w in enumerate(widths):
                    slc.append((t, i * NT, w))
                    t += w
                out.append((n0, nw, slc))
                n0 += nw
            return out

        def emit_gate_dmas():
            # gate stripes issue on the Activation HWDGE queue, everything
            # else on SP: the two queues issue descriptors in parallel
            # (~650ns serial issue each), so the gate path isn't starved.
            for s in range(NGS):
                nc.scalar.dma_start(wg_sb[:, s], wgu_r[:, :, s * GS:(s + 1) * GS])

        def emit_up_dmas():
            for s in range(F // WS):
                nc.sync.dma_start(wu_sb[:, s],
                                  wgu_r[:, :, F + s * WS:F + (s + 1) * WS])

        def emit_down_dmas():
            for s in range(D // WS):
                nc.sync.dma_start(wd_sb[:, s], wd_r[:, :, s * WS:(s + 1) * WS])

        def emit_weight_dmas():
            emit_gate_dmas()
            emit_up_dmas()
            emit_down_dmas()

        def do_chunk(n0, nw, slc, first_chunk, last_chunk):
            x_sb = xpool.tile([128, 3, KD, NT], dt.float16, tag="x")
            for si, (t, p, w) in enumerate(slc):
                nc.sync.dma_start(x_sb[:, si, :, :w],
                                  xT_r[:, :, n0 + t:n0 + t + w])
            if first_chunk and not weights_outside:
                if spread_weights:
                    emit_gate_dmas()
                else:
                    emit_weight_dmas()
            gh = ghpool.tile([128, KF, NB], dt.float16, tag="gh")
            for phase in (0, 1):  # 0: gate+silu, 1: up+mul-in-place
                for mp in range(KF):
                    if (first_chunk and spread_weights and not weights_outside
                            and mp == KF // 2):
                        # spread the bulk weight streams: up half midway
                        # through the gate phase, down weights midway through
                        # the up phase (~14us of lead each, > the ~11.5us of
                        # data) — decongests the rep-boundary DMA burst
                        if phase == 0:
                            emit_up_dmas()
                        else:
                            emit_down_dmas()
                    ps = pspool.tile([128, NB], dt.float32, tag="ps",
                                     name=f"ps_{n0}_{phase}_{mp}")
                    wsel = gate_w(mp) if phase == 0 else up_w(mp)
                    slice_outer = first_chunk and phase == 0 and mp < 2
                    if slice_outer:
                        loop = [(k, si) for si in range(len(slc))
                                for k in range(KD)]
                    else:
                        loop = [(k, si) for k in range(KD)
                                for si in range(len(slc))]
                    for k, si in loop:
                        t, p, w = slc[si]
                        nc.tensor.matmul(
                            ps[:, p:p + w], lhsT=wsel[:, k],
                            rhs=x_sb[:, si, k, :w],
                            start=(k == 0), stop=(k == KD - 1))
                    for t, p, w in slc:
                        if phase == 0:
                            nc.scalar.activation(
                                gh[:, mp, t:t + w], ps[:, p:p + w],
                                mybir.ActivationFunctionType.Silu)
                        else:
                            nc.vector.tensor_mul(gh[:, mp, t:t + w],
                                                 gh[:, mp, t:t + w],
                                                 ps[:, p:p + w])
            for m in range(MD):
                pso = pspool.tile([128, NB], dt.float32, tag="ps",
                                  name=f"pso_{n0}_{m}")
                # last m of the last chunk runs slice-outer: slice s's
                # accumulation closes ~5us before the kernel end, so its
                # eviction+DMA pipeline under the remaining matmuls and the
                # serial tail is one ~350-token eviction.
                tail = last_chunk and m == MD - 1
                o_sb = opool.tile([128, NB], dt.float32, tag="o",
                                  name=f"o_{n0}_{m}")
                if tail:
                    for si, (t, p, w) in enumerate(slc):
                        for k in range(KF):
                            nc.tensor.matmul(
                                pso[:, p:p + w], lhsT=down_w(m)[:, k],
                                rhs=gh[:, k, t:t + w],
                                start=(k == 0), stop=(k == KF - 1))
                        nc.vector.tensor_copy(o_sb[:, t:t + w], pso[:, p:p + w])
                        dge = nc.sync if si % 2 == 0 else nc.scalar
                        dge.dma_start(outT_r[:, m, n0 + t:n0 + t + w],
                                      o_sb[:, t:t + w])
                else:
                    for k in range(KF):
                        for t, p, w in slc:
                            nc.tensor.matmul(
                                pso[:, p:p + w], lhsT=down_w(m)[:, k],
                                rhs=gh[:, k, t:t + w],
                                start=(k == 0), stop=(k == KF - 1))
                    for t, p, w in slc:
                        nc.vector.tensor_copy(o_sb[:, t:t + w], pso[:, p:p + w])
                        dge = nc.sync if m % 2 == 0 else nc.scalar
                        dge.dma_start(outT_r[:, m, n0 + t:n0 + t + w],
                                      o_sb[:, t:t + w])

        def body():
            chunks = chunk_list()
            for i, (n0, nw, slc) in enumerate(chunks):
                do_chunk(n0, nw, slc, i == 0, i == len(chunks) - 1)

        if weights_outside:
            emit_weight_dmas()
        if hw_loop:
            with tc.For_i(0, hw_loop, 1):
                body()
        else:
            body()
    nc.finalize()
    return nc


def build_nc_v7nw(C, hw_loop=0):
    """v7 with weight DMAs hoisted out of the hw_loop (microbench: isolates
    the per-rep 12MB weight re-stream from the loop differential)."""
    return build_nc_v7(C, hw_loop=hw_loop, weights_outside=True)


def route(x, expert_indices):
    """Sort tokens by expert; return (order, counts, capacity C)."""
    idx = np.asarray(expert_indices)
    order = np.argsort(idx, kind="stable")
    counts = np.bincount(idx, minlength=E).astype(np.int64)
    C = max(NT, int(-(-counts.max() // 8) * 8))
    return order, counts, C


def make_in_maps(x, expert_indices, gate_up_weight, down_weight):
    order, counts, C = route(x, expert_indices)
    x_sorted = np.asarray(x, dtype=np.float32)[order]
    offs = np.zeros(E + 1, dtype=np.int64)
    np.cumsum(counts, out=offs[1:])
    wguT = np.ascontiguousarray(
        np.transpose(np.asarray(gate_up_weight), (0, 2, 1))).astype(F16)
    wdT = np.ascontiguousarray(
        np.transpose(np.asarray(down_weight), (0, 2, 1))).astype(F16)
    in_maps = []
    for e in range(E):
        xe = np.zeros((C, D), dtype=np.float32)
        xe[: counts[e]] = x_sorted[offs[e]: offs[e + 1]]
        in_maps.append({
            "xT": np.ascontiguousarray(xe.T).astype(F16),
            "wguT": wguT[e],
            "wdT": wdT[e],
        })
    return in_maps, order, counts, C


def assemble_output(results, order, counts):
    T = int(counts.sum())
    out = np.empty((T, D), dtype=np.float32)
    offs = np.zeros(E + 1, dtype=np.int64)
    np.cumsum(counts, out=offs[1:])
    sorted_out = np.empty((T, D), dtype=np.float32)
    for e in range(E):
        sorted_out[offs[e]: offs[e + 1]] = results[e]["outT"].T[: counts[e]]
    out[order] = sorted_out
    return out


def kernel(x, expert_indices, gate_up_weight, down_weight):
    in_maps, order, counts, C = make_in_maps(
        x, expert_indices, gate_up_weight, down_weight)
    nc = get_nc(C)
    res = run_bass_kernel_spmd(nc, in_maps, core_ids=list(range(E)))
    return assemble_output(res.results, order, counts)



# revision 31
# speedup vs baseline: 1.0021x; 1.0021x over previous
"""MoE batched-experts kernel for Trainium2 (8 NeuronCores, expert-parallel).

Contract: kernel(**inputs) takes the FULL unsharded inputs
  x:              [T, D]      float32   (T=16384, D=1024)
  expert_indices: [T]         int32     (values in [0, 8))
  gate_up_weight: [E, 2F, D]  float32   (E=8, F=2048)
  down_weight:    [E, D, F]   float32
and returns the FULL output [T, D] float32:
  per token t with expert e: h = silu(x @ gu[e,:F].T) * (x @ gu[e,F:].T);
  out = h @ dw[e].T

Strategy: expert-parallel. The host routes (sorts) tokens by expert, pads
each expert's group to a common capacity C (max count rounded up to 8), and
core e runs a dense FFN for expert e on its token group. All operands are
pre-transposed / pre-cast to fp16 on the host (same PE rate as bf16, 8x the
mantissa: rel-err ~6e-4 vs ~4.5e-3) so the device kernel is pure matmul +
silu*mul with no on-chip transposes:
  core e computes outT = w_d @ (silu(w_gT.T @ xT) * (w_uT.T @ xT))
with xT [D, C], producing outT [D, C] (bf16 in the shipped build_nc_v9:
halves writeback DMA, rel-err 3.4e-3 vs the 2e-2 gate); the host upcasts,
transposes back and unpermutes.
"""

import numpy as np
import ml_dtypes

import concourse.bass as bass
import concourse.mybir as mybir
from concourse import bacc
from concourse.tile import TileContext
from concourse.bass import ts, ds
from concourse.bass_utils import run_bass_kernel_spmd
from contextlib import ExitStack

BF16 = ml_dtypes.bfloat16
F16 = np.float16  # operand dtype for matmuls: same PE rate as bf16, 8x mantissa

D = 1024      # d_model
F = 2048      # d_ff
F2 = 2 * F    # gate+up
E = 8         # experts == cores
KD = D // 128   # 8  k-tiles over d_model
KF = F // 128   # 16 k-tiles over d_ff
MD = D // 128   # 8  m-tiles over d_model (output)
NT = 512        # token chunk (one PSUM bank at fp32)

_nc_cache = {}


def build_nc(C, repeats=1, hw_loop=0):
    """Build the per-core dense-FFN Bass program for token capacity C.

    repeats>1 re-emits the whole compute body (unrolled); hw_loop>0 wraps the
    body in a hardware For_i loop. Both are timing aids: slope of time vs
    repetition count isolates true exec time from dispatch overhead."""
    nc = bacc.Bacc("TRN2", target_bir_lowering=False, debug=False, num_devices=E)
    dt = mybir.dt
    xT = nc.dram_tensor("xT", [D, C], dt.bfloat16, kind="ExternalInput")
    wgu = nc.dram_tensor("wguT", [D, F2], dt.bfloat16, kind="ExternalInput")
    wd = nc.dram_tensor("wdT", [F, D], dt.bfloat16, kind="ExternalInput")
    outT = nc.dram_tensor("outT", [D, C], dt.float32, kind="ExternalOutput")

    with TileContext(nc) as tc, ExitStack() as ctx:
        wpool = ctx.enter_context(tc.tile_pool(name="weights", bufs=1))
        wgu_sb = wpool.tile([128, KD, F2], dt.bfloat16, tag="wgu")
        nc.sync.dma_start(wgu_sb[:], wgu.rearrange("(k p) f -> p k f", p=128))
        wd_sb = wpool.tile([128, KF, D], dt.bfloat16, tag="wd")
        nc.sync.dma_start(wd_sb[:], wd.rearrange("(k p) f -> p k f", p=128))

        xpool = ctx.enter_context(tc.tile_pool(name="x", bufs=2))
        hpool = ctx.enter_context(tc.tile_pool(name="h", bufs=2))
        spool = ctx.enter_context(tc.tile_pool(name="silu", bufs=4))
        opool = ctx.enter_context(tc.tile_pool(name="o", bufs=4))
        pg = ctx.enter_context(tc.tile_pool(name="pg", bufs=2, space="PSUM"))
        pu = ctx.enter_context(tc.tile_pool(name="pu", bufs=2, space="PSUM"))
        po = ctx.enter_context(tc.tile_pool(name="po", bufs=2, space="PSUM"))

        xT_r = xT.rearrange("(k p) t -> p k t", p=128)
        outT_r = outT.rearrange("(m p) t -> p m t", p=128)

        def body():
            for n0 in [i for _ in range(repeats) for i in range(0, C, NT)]:
                nt = min(NT, C - n0)
                x_sb = xpool.tile([128, KD, NT], dt.bfloat16, tag="x")
                nc.sync.dma_start(x_sb[:, :, :nt], xT_r[:, :, n0:n0 + nt])
                h_sb = hpool.tile([128, KF, NT], dt.bfloat16, tag="h")
                for mp in range(KF):
                    psg = pg.tile([128, NT], dt.float32, tag="pg")
                    for k in range(KD):
                        nc.tensor.matmul(
                            psg[:, :nt], lhsT=wgu_sb[:, k, ts(mp, 128)],
                            rhs=x_sb[:, k, :nt], start=(k == 0), stop=(k == KD - 1))
                    psu = pu.tile([128, NT], dt.float32, tag="pu")
                    for k in range(KD):
                        nc.tensor.matmul(
                            psu[:, :nt], lhsT=wgu_sb[:, k, ds(F + mp * 128, 128)],
                            rhs=x_sb[:, k, :nt], start=(k == 0), stop=(k == KD - 1))
                    sil = spool.tile([128, NT], dt.bfloat16, tag="sil")
                    nc.scalar.activation(sil[:, :nt], psg[:, :nt],
                                         mybir.ActivationFunctionType.Silu)
                    nc.vector.tensor_mul(h_sb[:, mp, :nt], sil[:, :nt], psu[:, :nt])
                for m in range(MD):
                    pso = po.tile([128, NT], dt.float32, tag="po")
                    for k in range(KF):
                        nc.tensor.matmul(
                            pso[:, :nt], lhsT=wd_sb[:, k, ts(m, 128)],
                            rhs=h_sb[:, k, :nt], start=(k == 0), stop=(k == KF - 1))
                    o_sb = opool.tile([128, NT], dt.float32, tag="o")
                    nc.vector.tensor_copy(o_sb[:, :nt], pso[:, :nt])
                    nc.sync.dma_start(outT_r[:, m, n0:n0 + nt], o_sb[:, :nt])

        if hw_loop:
            with tc.For_i(0, hw_loop, 1):
                body()
        else:
            body()
    nc.finalize()
    return nc


def build_nc_wide(C, hw_loop=0):
    """Variant: 1024-token compute chunks with [128,1024] PSUM tiles.

    - halves ACT/DVE eviction instruction count (wide silu/mul)
    - consecutive matmuls share the same lhsT (LDW dedup opportunity)
    - PSUM banks: pg 2x2 + pu 1x2 + po 2x1 = 8
    """
    nc = bacc.Bacc("TRN2", target_bir_lowering=False, debug=False, num_devices=E)
    dt = mybir.dt
    NW = 1024
    xT = nc.dram_tensor("xT", [D, C], dt.bfloat16, kind="ExternalInput")
    wgu = nc.dram_tensor("wguT", [D, F2], dt.bfloat16, kind="ExternalInput")
    wd = nc.dram_tensor("wdT", [F, D], dt.bfloat16, kind="ExternalInput")
    outT = nc.dram_tensor("outT", [D, C], dt.float32, kind="ExternalOutput")

    with TileContext(nc) as tc, ExitStack() as ctx:
        wpool = ctx.enter_context(tc.tile_pool(name="weights", bufs=1))
        wgu_sb = wpool.tile([128, KD, F2], dt.bfloat16, tag="wgu")
        nc.sync.dma_start(wgu_sb[:], wgu.rearrange("(k p) f -> p k f", p=128))
        wd_sb = wpool.tile([128, KF, D], dt.bfloat16, tag="wd")
        nc.sync.dma_start(wd_sb[:], wd.rearrange("(k p) f -> p k f", p=128))

        xpool = ctx.enter_context(tc.tile_pool(name="x", bufs=1))
        hpool = ctx.enter_context(tc.tile_pool(name="h", bufs=3))
        spool = ctx.enter_context(tc.tile_pool(name="silu", bufs=3))
        opool = ctx.enter_context(tc.tile_pool(name="o", bufs=4))
        pg = ctx.enter_context(tc.tile_pool(name="pg", bufs=2, space="PSUM"))
        pu = ctx.enter_context(tc.tile_pool(name="pu", bufs=1, space="PSUM"))
        po = ctx.enter_context(tc.tile_pool(name="po", bufs=2, space="PSUM"))

        xT_r = xT.rearrange("(k p) t -> p k t", p=128)
        outT_r = outT.rearrange("(m p) t -> p m t", p=128)

        def do_chunk(n0, nw):
            # nw tokens starting at n0; nw in {1024, C % 1024}
            nh = (nw + NT - 1) // NT  # h sub-chunks of <=512
            x_sb = xpool.tile([128, KD, NW], dt.bfloat16, tag="x")
            nc.sync.dma_start(x_sb[:, :, :nw], xT_r[:, :, n0:n0 + nw])
            h_sbs = [hpool.tile([128, KF, NT], dt.bfloat16, tag="h",
                                name=f"h_{n0}_{s}")
                     for s in range(nh)]
            for mp in range(KF):
                psg = pg.tile([128, NW], dt.float32, tag="pg")
                for k in range(KD):
                    for s in range(nh):
                        w = min(NT, nw - s * NT)
                        nc.tensor.matmul(
                            psg[:, s * NT:s * NT + w],
                            lhsT=wgu_sb[:, k, ts(mp, 128)],
                            rhs=x_sb[:, k, s * NT:s * NT + w],
                            start=(k == 0), stop=(k == KD - 1))
                psu = pu.tile([128, NW], dt.float32, tag="pu")
                for k in range(KD):
                    for s in range(nh):
                        w = min(NT, nw - s * NT)
                        nc.tensor.matmul(
                            psu[:, s * NT:s * NT + w],
                            lhsT=wgu_sb[:, k, ds(F + mp * 128, 128)],
                            rhs=x_sb[:, k, s * NT:s * NT + w],
                            start=(k == 0), stop=(k == KD - 1))
                sil = spool.tile([128, NW], dt.bfloat16, tag="sil")
                nc.scalar.activation(sil[:, :nw], psg[:, :nw],
                                     mybir.ActivationFunctionType.Silu)
                for s in range(nh):
                    w = min(NT, nw - s * NT)
                    nc.vector.tensor_mul(h_sbs[s][:, mp, :w],
                                         sil[:, s * NT:s * NT + w],
                                         psu[:, s * NT:s * NT + w])
            for m in range(MD):
                for s in range(nh):
                    w = min(NT, nw - s * NT)
                    pso = po.tile([128, NT], dt.float32, tag="po")
                    for k in range(KF):
                        nc.tensor.matmul(
                            pso[:, :w], lhsT=wd_sb[:, k, ts(m, 128)],
                            rhs=h_sbs[s][:, k, :w],
                            start=(k == 0), stop=(k == KF - 1))
                    o_sb = opool.tile([128, NT], dt.float32, tag="o")
                    nc.vector.tensor_copy(o_sb[:, :w], pso[:, :w])
                    nc.sync.dma_start(outT_r[:, m, n0 + s * NT:n0 + s * NT + w],
                                      o_sb[:, :w])

        def body():
            for n0 in range(0, C, NW):
                do_chunk(n0, min(NW, C - n0))

        if hw_loop:
            with tc.For_i(0, hw_loop, 1):
                body()
        else:
            body()
    nc.finalize()
    return nc


def get_nc(C):
    # build_nc_v7: TimelineSim 355.9us vs build_nc_big's 365.3us; fewer DMA
    # descriptors (~26 vs 184; each costs ~650ns of serial HWDGE issue) and
    # 1:3 LDW:MM in the down phase (vs 1:1).  HW loop-differential measures
    # ~433-447us/rep — at the machine's sustained matmul-stream limit: a
    # pure-MM microbench with zero DMA/deps measures 426us for the same
    # column count (PE effectively ~1.9GHz under sustained load, not 2.4).
    # fp8 DoubleRow (1.44x PE) was evaluated and rejected: e4m3 operands
    # give rel-err ~7e-2 on this problem vs the 2e-2 gate (measured in
    # numpy emulation; error is mantissa-limited, scaling cannot fix it).
    if C not in _nc_cache:
        _nc_cache[C] = BUILD(C)
    return _nc_cache[C]


def build_nc_big(C, hw_loop=0):
    """Variant: 1536-token chunks ([128,1536] 3-bank PSUM tiles).

    Streams 3x512 tokens per weight load (LDW count 1920 -> ~768), evicts
    gate via silu into a chunk-resident SBUF tensor, then multiplies the up
    projection into it in place. PSUM: pp 2x3 + po 2x1 = 8 banks.
    """
    nc = bacc.Bacc("TRN2", target_bir_lowering=False, debug=False, num_devices=E)
    dt = mybir.dt
    NB = 1536
    xT = nc.dram_tensor("xT", [D, C], dt.float16, kind="ExternalInput")
    wgu = nc.dram_tensor("wguT", [D, F2], dt.float16, kind="ExternalInput")
    wd = nc.dram_tensor("wdT", [F, D], dt.float16, kind="ExternalInput")
    outT = nc.dram_tensor("outT", [D, C], dt.float32, kind="ExternalOutput")

    with TileContext(nc) as tc, ExitStack() as ctx:
        # per-k weight tiles with separate DMAs; the first chunk's x tiles
        # are DMA'd BEFORE the weights (see do_chunk) so the PE's first
        # matmul group is gated on ~4 MB, not the full 16 MB input set.
        wpool = ctx.enter_context(tc.tile_pool(name="weights", bufs=1))
        wgu_k = [wpool.tile([128, F2], dt.float16, tag=f"wgu{k}",
                            name=f"wgu{k}") for k in range(KD)]
        wd_k = [wpool.tile([128, D], dt.float16, tag=f"wd{k}",
                           name=f"wd{k}") for k in range(KF)]

        xpool = ctx.enter_context(tc.tile_pool(name="x", bufs=1))
        ghpool = ctx.enter_context(tc.tile_pool(name="gh", bufs=1))
        opool = ctx.enter_context(tc.tile_pool(name="o", bufs=4))
        pp = ctx.enter_context(tc.tile_pool(name="pp", bufs=2, space="PSUM"))
        po = ctx.enter_context(tc.tile_pool(name="po", bufs=2, space="PSUM"))

        xT_r = xT.rearrange("(k p) t -> p k t", p=128)
        outT_r = outT.rearrange("(m p) t -> p m t", p=128)

        def slices(nw):
            return [(s, min(NT, nw - s)) for s in range(0, nw, NT)]

        def do_chunk(n0, nw):
            x_sb = xpool.tile([128, KD, NB], dt.float16, tag="x")
            if first[0]:
                first[0] = False
                # interleave x and gate/up weight k-tiles so the first
                # matmul group's operands stream in consumption order
                for k in range(KD):
                    nc.sync.dma_start(x_sb[:, k, :nw], xT_r[:, k, n0:n0 + nw])
                    # first 512 f-columns land first so the k-th LDW of the
                    # first gate group unblocks after ~0.4 MB, not 1 MB
                    nc.sync.dma_start(wgu_k[k][:, :NT],
                                      wgu[k * 128:(k + 1) * 128, :NT])
                    nc.sync.dma_start(wgu_k[k][:, NT:],
                                      wgu[k * 128:(k + 1) * 128, NT:])
                for k in range(KF):
                    nc.sync.dma_start(wd_k[k][:], wd[k * 128:(k + 1) * 128, :])
            else:
                for k in range(KD):
                    nc.sync.dma_start(x_sb[:, k, :nw], xT_r[:, k, n0:n0 + nw])
            gh = ghpool.tile([128, KF, NB], dt.float16, tag="gh")
            for phase in (0, 1):  # 0: gate+silu, 1: up+mul-in-place
                for mp in range(KF):
                    ps = pp.tile([128, NB], dt.float32, tag="pp",
                                 name=f"ps_{n0}_{phase}_{mp}")
                    f0 = mp * 128 if phase == 0 else F + mp * 128
                    for k in range(KD):
                        for s, w in slices(nw):
                            nc.tensor.matmul(
                                ps[:, s:s + w],
                                lhsT=wgu_k[k][:, ds(f0, 128)],
                                rhs=x_sb[:, k, s:s + w],
                                start=(k == 0), stop=(k == KD - 1))
                    if phase == 0:
                        nc.scalar.activation(gh[:, mp, :nw], ps[:, :nw],
                                             mybir.ActivationFunctionType.Silu)
                    else:
                        nc.vector.tensor_mul(gh[:, mp, :nw], gh[:, mp, :nw],
                                             ps[:, :nw])
            for m in range(MD):
                for s, w in slices(nw):
                    pso = po.tile([128, NT], dt.float32, tag="po",
                                  name=f"pso_{n0}_{m}_{s}")
                    for k in range(KF):
                        nc.tensor.matmul(
                            pso[:, :w], lhsT=wd_k[k][:, ts(m, 128)],
                            rhs=gh[:, k, s:s + w],
                            start=(k == 0), stop=(k == KF - 1))
                    o_sb = opool.tile([128, NT], dt.float32, tag="o",
                                      name=f"o_{n0}_{m}_{s}")
                    nc.vector.tensor_copy(o_sb[:, :w], pso[:, :w])
                    nc.sync.dma_start(outT_r[:, m, n0 + s:n0 + s + w],
                                      o_sb[:, :w])

        first = [True]

        def body():
            # smallest chunk first: the cold-start stall is gated on the
            # first chunk's x DMA, so lead with the cheapest one
            chunks = [(n0, min(NB, C - n0)) for n0 in range(0, C, NB)]
            chunks.sort(key=lambda c: c[1])
            for n0, nw in chunks:
                do_chunk(n0, nw)



        if hw_loop:
            with tc.For_i(0, hw_loop, 1):
                body()
        else:
            body()
    nc.finalize()
    return nc


def build_nc_v3(C, hw_loop=0):
    """Tuned variant of build_nc_big:

    - weight DMAs striped in consumption order (512-col stripes across all
      k-tiles, gate half first, then up half, then wd) so the first gate
      phase is never DMA-starved;
    - remainder-chunk token slices equalized (e.g. 568 -> 284+284, not
      512+56) so no runt matmuls pay the per-MM issue floor;
    - down phase restructured m -> k -> s with a full-width [128, NB] PSUM
      tile from the shared pool: 1 LDW per 3 matmuls instead of 1:1.
    PSUM: ps pool 2x3 banks = 6 of 8 banks.
    """
    nc = bacc.Bacc("TRN2", target_bir_lowering=False, debug=False, num_devices=E)
    dt = mybir.dt
    NB = 1536
    xT = nc.dram_tensor("xT", [D, C], dt.float16, kind="ExternalInput")
    wgu = nc.dram_tensor("wguT", [D, F2], dt.float16, kind="ExternalInput")
    wd = nc.dram_tensor("wdT", [F, D], dt.float16, kind="ExternalInput")
    outT = nc.dram_tensor("outT", [D, C], dt.float32, kind="ExternalOutput")

    with TileContext(nc) as tc, ExitStack() as ctx:
        wpool = ctx.enter_context(tc.tile_pool(name="weights", bufs=1))
        wgu_k = [wpool.tile([128, F2], dt.float16, tag=f"wgu{k}",
                            name=f"wgu{k}") for k in range(KD)]
        wd_k = [wpool.tile([128, D], dt.float16, tag=f"wd{k}",
                           name=f"wd{k}") for k in range(KF)]

        xpool = ctx.enter_context(tc.tile_pool(name="x", bufs=1))
        ghpool = ctx.enter_context(tc.tile_pool(name="gh", bufs=1))
        opool = ctx.enter_context(tc.tile_pool(name="o", bufs=2))
        pspool = ctx.enter_context(tc.tile_pool(name="ps", bufs=2, space="PSUM"))

        xT_r = xT.rearrange("(k p) t -> p k t", p=128)
        outT_r = outT.rearrange("(m p) t -> p m t", p=128)

        def slices(nw):
            ns = (nw + NT - 1) // NT
            w = -(-nw // ns)  # equal widths, last may be smaller by <ns
            return [(s, min(w, nw - s)) for s in range(0, nw, w)]

        def chunk_list():
            chunks = []
            rem = C
            while rem > 0:
                take = NB if rem >= NB else rem
                chunks.append(take)
                rem -= take
            sizes = sorted(chunks)  # smallest first: cheapest cold start
            offs = []
            n0 = 0
            for s in sizes:
                offs.append((n0, s))
                n0 += s
            return offs

        def emit_weight_dmas():
            # gate half, then up half: 512-col stripes across all k-tiles in
            # the order the first gate phase consumes them
            for half in (0, F):
                for c0 in range(half, half + F, 512):
                    for k in range(KD):
                        nc.sync.dma_start(wgu_k[k][:, c0:c0 + 512],
                                          wgu[k * 128:(k + 1) * 128, c0:c0 + 512])
            for c0 in range(0, D, 512):
                for k in range(KF):
                    nc.sync.dma_start(wd_k[k][:, c0:c0 + 512],
                                      wd[k * 128:(k + 1) * 128, c0:c0 + 512])

        def do_chunk(n0, nw, first_chunk):
            x_sb = xpool.tile([128, KD, NB], dt.float16, tag="x")
            if first_chunk:
                for k in range(KD):
                    nc.sync.dma_start(x_sb[:, k, :nw], xT_r[:, k, n0:n0 + nw])
                emit_weight_dmas()
            else:
                for k in range(KD):
                    nc.sync.dma_start(x_sb[:, k, :nw], xT_r[:, k, n0:n0 + nw])
            gh = ghpool.tile([128, KF, NB], dt.float16, tag="gh")
            for phase in (0, 1):  # 0: gate+silu, 1: up+mul-in-place
                for mp in range(KF):
                    ps = pspool.tile([128, NB], dt.float32, tag="ps",
                                     name=f"ps_{n0}_{phase}_{mp}")
                    f0 = mp * 128 if phase == 0 else F + mp * 128
                    for k in range(KD):
                        for s, w in slices(nw):
                            nc.tensor.matmul(
                                ps[:, s:s + w],
                                lhsT=wgu_k[k][:, ds(f0, 128)],
                                rhs=x_sb[:, k, s:s + w],
                                start=(k == 0), stop=(k == KD - 1))
                    if phase == 0:
                        nc.scalar.activation(gh[:, mp, :nw], ps[:, :nw],
                                             mybir.ActivationFunctionType.Silu)
                    else:
                        nc.vector.tensor_mul(gh[:, mp, :nw], gh[:, mp, :nw],
                                             ps[:, :nw])
            for m in range(MD):
                pso = pspool.tile([128, NB], dt.float32, tag="ps",
                                  name=f"pso_{n0}_{m}")
                for k in range(KF):
                    for s, w in slices(nw):
                        nc.tensor.matmul(
                            pso[:, s:s + w], lhsT=wd_k[k][:, ts(m, 128)],
                            rhs=gh[:, k, s:s + w],
                            start=(k == 0), stop=(k == KF - 1))
                o_sb = opool.tile([128, NB], dt.float32, tag="o",
                                  name=f"o_{n0}_{m}")
                nc.vector.tensor_copy(o_sb[:, :nw], pso[:, :nw])
                nc.sync.dma_start(outT_r[:, m, n0:n0 + nw], o_sb[:, :nw])

        def body():
            for i, (n0, nw) in enumerate(chunk_list()):
                do_chunk(n0, nw, i == 0)

        if hw_loop:
            with tc.For_i(0, hw_loop, 1):
                body()
        else:
            body()
    nc.finalize()
    return nc


def build_nc_v4(C, hw_loop=0):
    """v3 + better chunking and cold-start:

    - chunks [first(1 slice), 1536*q (3x512 slices), last(1 slice)]: the
      first/last remainder chunks absorb C%512 as two ~equal >=256-token
      1-slice chunks, so every matmul is >=256 wide (no runt-MM issue-floor
      waste), every PSUM write is bank-aligned, and both the cold-start x
      DMA and the serial eviction tail after the last matmul are small;
    - the first 512 gate-weight columns stream in 128-col pieces so the
      first matmul group unblocks after ~0.3 MB.
    """
    nc = bacc.Bacc("TRN2", target_bir_lowering=False, debug=False, num_devices=E)
    dt = mybir.dt
    NB = 1536
    xT = nc.dram_tensor("xT", [D, C], dt.float16, kind="ExternalInput")
    wgu = nc.dram_tensor("wguT", [D, F2], dt.float16, kind="ExternalInput")
    wd = nc.dram_tensor("wdT", [F, D], dt.float16, kind="ExternalInput")
    outT = nc.dram_tensor("outT", [D, C], dt.float32, kind="ExternalOutput")

    with TileContext(nc) as tc, ExitStack() as ctx:
        wpool = ctx.enter_context(tc.tile_pool(name="weights", bufs=1))
        wgu_k = [wpool.tile([128, F2], dt.float16, tag=f"wgu{k}",
                            name=f"wgu{k}") for k in range(KD)]
        wd_k = [wpool.tile([128, D], dt.float16, tag=f"wd{k}",
                           name=f"wd{k}") for k in range(KF)]

        xpool = ctx.enter_context(tc.tile_pool(name="x", bufs=1))
        ghpool = ctx.enter_context(tc.tile_pool(name="gh", bufs=1))
        opool = ctx.enter_context(tc.tile_pool(name="o", bufs=2))
        pspool = ctx.enter_context(tc.tile_pool(name="ps", bufs=2, space="PSUM"))

        xT_r = xT.rearrange("(k p) t -> p k t", p=128)
        outT_r = outT.rearrange("(m p) t -> p m t", p=128)

        def chunk_list():
            """[(n0, nw, [slice widths])]; each chunk is 1 slice of any
            width, or all-512 slices (bank alignment for free)."""
            ns = -(-C // NT)
            if ns <= 3:
                w = -(-C // ns)
                widths = [min(w, C - i * w) for i in range(ns)]
                groups = [[wi] for wi in widths]
            else:
                slack = NT * ns - C
                wf = NT - (slack + 1) // 2
                wl = NT - slack // 2
                mid = [NT] * (ns - 2)
                groups = [[wf]] + [mid[i:i + 3] for i in range(0, len(mid), 3)] \
                    + [[wl]]
            out, n0 = [], 0
            for g in groups:
                out.append((n0, sum(g), g))
                n0 += sum(g)
            return out

        def emit_weight_dmas():
            for c0 in range(0, 512, 128):  # first gate stripe: fine-grained
                for k in range(KD):
                    nc.sync.dma_start(wgu_k[k][:, c0:c0 + 128],
                                      wgu[k * 128:(k + 1) * 128, c0:c0 + 128])
            for half in (0, F):
                for c0 in range(half, half + F, 512):
                    if c0 == 0:
                        continue  # already emitted fine-grained
                    for k in range(KD):
                        nc.sync.dma_start(wgu_k[k][:, c0:c0 + 512],
                                          wgu[k * 128:(k + 1) * 128, c0:c0 + 512])
            for c0 in range(0, D, 512):
                for k in range(KF):
                    nc.sync.dma_start(wd_k[k][:, c0:c0 + 512],
                                      wd[k * 128:(k + 1) * 128, c0:c0 + 512])

        def do_chunk(n0, nw, widths, first_chunk):
            x_sb = xpool.tile([128, KD, NB], dt.float16, tag="x")
            for k in range(KD):
                nc.sync.dma_start(x_sb[:, k, :nw], xT_r[:, k, n0:n0 + nw])
            if first_chunk:
                emit_weight_dmas()
            slc = []
            s = 0
            for w in widths:
                slc.append((s, w))
                s += w
            gh = ghpool.tile([128, KF, NB], dt.float16, tag="gh")
            for phase in (0, 1):  # 0: gate+silu, 1: up+mul-in-place
                for mp in range(KF):
                    ps = pspool.tile([128, NB], dt.float32, tag="ps",
                                     name=f"ps_{n0}_{phase}_{mp}")
                    f0 = mp * 128 if phase == 0 else F + mp * 128
                    for k in range(KD):
                        for s, w in slc:
                            nc.tensor.matmul(
                                ps[:, s:s + w],
                                lhsT=wgu_k[k][:, ds(f0, 128)],
                                rhs=x_sb[:, k, s:s + w],
                                start=(k == 0), stop=(k == KD - 1))
                    if phase == 0:
                        nc.scalar.activation(gh[:, mp, :nw], ps[:, :nw],
                                             mybir.ActivationFunctionType.Silu)
                    else:
                        nc.vector.tensor_mul(gh[:, mp, :nw], gh[:, mp, :nw],
                                             ps[:, :nw])
            for m in range(MD):
                pso = pspool.tile([128, NB], dt.float32, tag="ps",
                                  name=f"pso_{n0}_{m}")
                for k in range(KF):
                    for s, w in slc:
                        nc.tensor.matmul(
                            pso[:, s:s + w], lhsT=wd_k[k][:, ts(m, 128)],
                            rhs=gh[:, k, s:s + w],
                            start=(k == 0), stop=(k == KF - 1))
                o_sb = opool.tile([128, NB], dt.float32, tag="o",
                                  name=f"o_{n0}_{m}")
                nc.vector.tensor_copy(o_sb[:, :nw], pso[:, :nw])
                nc.sync.dma_start(outT_r[:, m, n0:n0 + nw], o_sb[:, :nw])

        def body():
            for i, (n0, nw, widths) in enumerate(chunk_list()):
                do_chunk(n0, nw, widths, i == 0)

        if hw_loop:
            with tc.For_i(0, hw_loop, 1):
                body()
        else:
            body()
    nc.finalize()
    return nc


def build_nc_v5(C, hw_loop=0):
    """Bank-safe tuned variant (the successor of build_nc_big):

    - chunks [rem (2 equal slices), 1536 (3x512)...]: every matmul is
      >=256 tokens wide and every PSUM write sits in its own bank-aligned
      512-column slot (slice i of a chunk lives at psum column 512*i);
    - silu/mul/eviction/out-DMA run per-slice, so the serial tail after the
      very last matmul is one 512-wide eviction, not a whole 1536 chunk;
    - weight DMAs stream in consumption order (gate stripes across k first,
      the first stripe in 128-col pieces, then up half, then down weights);
    - down phase is m -> k -> slice with a full-width PSUM tile: one
      weight load per 3 matmuls.
    PSUM: shared ps pool 2x3 banks = 6 of 8 banks.
    """
    nc = bacc.Bacc("TRN2", target_bir_lowering=False, debug=False, num_devices=E)
    dt = mybir.dt
    NB = 1536
    xT = nc.dram_tensor("xT", [D, C], dt.float16, kind="ExternalInput")
    wgu = nc.dram_tensor("wguT", [D, F2], dt.float16, kind="ExternalInput")
    wd = nc.dram_tensor("wdT", [F, D], dt.float16, kind="ExternalInput")
    outT = nc.dram_tensor("outT", [D, C], dt.float32, kind="ExternalOutput")

    with TileContext(nc) as tc, ExitStack() as ctx:
        wpool = ctx.enter_context(tc.tile_pool(name="weights", bufs=1))
        wgu_k = [wpool.tile([128, F2], dt.float16, tag=f"wgu{k}",
                            name=f"wgu{k}") for k in range(KD)]
        wd_k = [wpool.tile([128, D], dt.float16, tag=f"wd{k}",
                           name=f"wd{k}") for k in range(KF)]

        xpool = ctx.enter_context(tc.tile_pool(name="x", bufs=1))
        ghpool = ctx.enter_context(tc.tile_pool(name="gh", bufs=1))
        opool = ctx.enter_context(tc.tile_pool(name="o", bufs=2))
        pspool = ctx.enter_context(tc.tile_pool(name="ps", bufs=2, space="PSUM"))

        xT_r = xT.rearrange("(k p) t -> p k t", p=128)
        outT_r = outT.rearrange("(m p) t -> p m t", p=128)

        def chunk_list():
            """[(n0, nw, [(tok_off, psum_off, w), ...])] — remainder first
            (2 equal slices), then full 1536 chunks (3x512)."""
            rem = C % NB
            chunks = []
            if rem:
                if rem <= NT:
                    w0 = (rem + 1) // 2
                    widths = [w0, rem - w0] if rem - w0 else [w0]
                elif rem <= 2 * NT:
                    w0 = (rem + 1) // 2
                    widths = [w0, rem - w0]
                else:
                    w0 = (rem + 2) // 3
                    widths = [w0, w0, rem - 2 * w0]
                chunks.append(widths)
            chunks += [[NT, NT, NT]] * (C // NB)
            out, n0 = [], 0
            for widths in chunks:
                slc, t = [], 0
                for i, w in enumerate(widths):
                    slc.append((t, i * NT, w))
                    t += w
                out.append((n0, sum(widths), slc))
                n0 += sum(widths)
            return out

        def emit_weight_dmas():
            for c0 in range(0, 512, 128):  # first gate stripe: fine-grained
                for k in range(KD):
                    nc.sync.dma_start(wgu_k[k][:, c0:c0 + 128],
                                      wgu[k * 128:(k + 1) * 128, c0:c0 + 128])
            for half in (0, F):
                for c0 in range(half, half + F, 512):
                    if c0 == 0:
                        continue  # emitted fine-grained above
                    for k in range(KD):
                        nc.sync.dma_start(wgu_k[k][:, c0:c0 + 512],
                                          wgu[k * 128:(k + 1) * 128, c0:c0 + 512])
            for c0 in range(0, D, 512):
                for k in range(KF):
                    nc.sync.dma_start(wd_k[k][:, c0:c0 + 512],
                                      wd[k * 128:(k + 1) * 128, c0:c0 + 512])

        def do_chunk(n0, nw, slc, first_chunk):
            x_sb = xpool.tile([128, KD, NB], dt.float16, tag="x")
            for k in range(KD):
                nc.sync.dma_start(x_sb[:, k, :nw], xT_r[:, k, n0:n0 + nw])
            if first_chunk:
                emit_weight_dmas()
            gh = ghpool.tile([128, KF, NB], dt.float16, tag="gh")
            for phase in (0, 1):  # 0: gate+silu, 1: up+mul-in-place
                for mp in range(KF):
                    ps = pspool.tile([128, NB], dt.float32, tag="ps",
                                     name=f"ps_{n0}_{phase}_{mp}")
                    f0 = mp * 128 if phase == 0 else F + mp * 128
                    for k in range(KD):
                        for t, p, w in slc:
                            nc.tensor.matmul(
                                ps[:, p:p + w],
                                lhsT=wgu_k[k][:, ds(f0, 128)],
                                rhs=x_sb[:, k, t:t + w],
                                start=(k == 0), stop=(k == KD - 1))
                    for t, p, w in slc:
                        if phase == 0:
                            nc.scalar.activation(
                                gh[:, mp, t:t + w], ps[:, p:p + w],
                                mybir.ActivationFunctionType.Silu)
                        else:
                            nc.vector.tensor_mul(gh[:, mp, t:t + w],
                                                 gh[:, mp, t:t + w],
                                                 ps[:, p:p + w])
            for m in range(MD):
                pso = pspool.tile([128, NB], dt.float32, tag="ps",
                                  name=f"pso_{n0}_{m}")
                for k in range(KF):
                    for t, p, w in slc:
                        nc.tensor.matmul(
                            pso[:, p:p + w], lhsT=wd_k[k][:, ts(m, 128)],
                            rhs=gh[:, k, t:t + w],
                            start=(k == 0), stop=(k == KF - 1))
                o_sb = opool.tile([128, NB], dt.float32, tag="o",
                                  name=f"o_{n0}_{m}")
                for t, p, w in slc:
                    nc.vector.tensor_copy(o_sb[:, t:t + w], pso[:, p:p + w])
                    nc.sync.dma_start(outT_r[:, m, n0 + t:n0 + t + w],
                                      o_sb[:, t:t + w])

        def body():
            for i, (n0, nw, slc) in enumerate(chunk_list()):
                do_chunk(n0, nw, slc, i == 0)

        if hw_loop:
            with tc.For_i(0, hw_loop, 1):
                body()
        else:
            body()
    nc.finalize()
    return nc


def build_nc_v6(C, hw_loop=0):
    """Uniform 2-slice chunks + few big DMAs + 3-deep PSUM pool.

    TimelineSim showed two costs the 1536-chunk builds pay: (a) each
    dma_start costs ~650ns of serial issue on the sync queue, so per-k /
    per-stripe descriptor spam delays the first matmul by ~8us; (b) with
    2x3-bank PSUM tiles the silu/mul round-trip doesn't fit the 2-buffer
    recycle window for narrow slices, stalling PE ~0.4us per group.

    Here: chunks are ceil(C/1024) near-equal sizes, each 2 bank-aligned
    slices -> PSUM tiles are [128,1024] (2 banks) and the pool holds 3
    bufs (6 banks): two full groups of recycle slack. Weights live in two
    monolithic SBUF tiles so each 512-col stripe (all k-tiles) is ONE
    descriptor, ordered gate-half, up-half, down; x streams one descriptor
    per slice.
    """
    nc = bacc.Bacc("TRN2", target_bir_lowering=False, debug=False, num_devices=E)
    dt = mybir.dt
    NBC = 1024
    xT = nc.dram_tensor("xT", [D, C], dt.float16, kind="ExternalInput")
    wgu = nc.dram_tensor("wguT", [D, F2], dt.float16, kind="ExternalInput")
    wd = nc.dram_tensor("wdT", [F, D], dt.float16, kind="ExternalInput")
    outT = nc.dram_tensor("outT", [D, C], dt.float32, kind="ExternalOutput")

    with TileContext(nc) as tc, ExitStack() as ctx:
        wpool = ctx.enter_context(tc.tile_pool(name="weights", bufs=1))
        wgu_sb = wpool.tile([128, KD, F2], dt.float16, tag="wgu")
        wd_sb = wpool.tile([128, KF, D], dt.float16, tag="wd")

        xpool = ctx.enter_context(tc.tile_pool(name="x", bufs=2))
        ghpool = ctx.enter_context(tc.tile_pool(name="gh", bufs=1))
        opool = ctx.enter_context(tc.tile_pool(name="o", bufs=3))
        pspool = ctx.enter_context(tc.tile_pool(name="ps", bufs=3, space="PSUM"))

        xT_r = xT.rearrange("(k p) t -> p k t", p=128)
        wgu_r = wgu.rearrange("(k p) f -> p k f", p=128)
        wd_r = wd.rearrange("(k p) m -> p k m", p=128)
        outT_r = outT.rearrange("(m p) t -> p m t", p=128)

        def chunk_list():
            """[(n0, nw, [(tok_off, psum_off, w), ...])] near-equal 2-slice
            chunks."""
            nch = -(-C // NBC)
            base, ext = divmod(C, nch)
            sizes = [base + (1 if i < ext else 0) for i in range(nch)]
            out, n0 = [], 0
            for nw in sizes:
                w0 = (nw + 1) // 2
                slc = [(0, 0, w0)]
                if nw - w0:
                    slc.append((w0, NT, nw - w0))
                out.append((n0, nw, slc))
                n0 += nw
            return out

        def emit_weight_dmas():
            # gate half first, leading 512 cols in two 256-col pieces so the
            # first matmul group unblocks after ~0.5 MB
            for c0, c1 in [(0, 256), (256, 512)] + [
                    (c, c + 512) for c in range(512, F, 512)]:
                nc.sync.dma_start(wgu_sb[:, :, c0:c1], wgu_r[:, :, c0:c1])
            for c0 in range(F, F2, 512):
                nc.sync.dma_start(wgu_sb[:, :, c0:c0 + 512],
                                  wgu_r[:, :, c0:c0 + 512])
            for c0 in range(0, D, 512):
                nc.sync.dma_start(wd_sb[:, :, c0:c0 + 512],
                                  wd_r[:, :, c0:c0 + 512])

        def do_chunk(n0, nw, slc, first_chunk):
            x_sb = xpool.tile([128, KD, NBC], dt.float16, tag="x")
            for t, p, w in slc:
                nc.sync.dma_start(x_sb[:, :, t:t + w], xT_r[:, :, n0 + t:n0 + t + w])
                if first_chunk and t == 0:
                    emit_weight_dmas()
            gh = ghpool.tile([128, KF, NBC], dt.float16, tag="gh")
            for phase in (0, 1):  # 0: gate+silu, 1: up+mul-in-place
                for mp in range(KF):
                    ps = pspool.tile([128, NBC], dt.float32, tag="ps",
                                     name=f"ps_{n0}_{phase}_{mp}")
                    f0 = mp * 128 if phase == 0 else F + mp * 128
                    for k in range(KD):
                        for t, p, w in slc:
                            nc.tensor.matmul(
                                ps[:, p:p + w],
                                lhsT=wgu_sb[:, k, ds(f0, 128)],
                                rhs=x_sb[:, k, t:t + w],
                                start=(k == 0), stop=(k == KD - 1))
                    for t, p, w in slc:
                        if phase == 0:
                            nc.scalar.activation(
                                gh[:, mp, t:t + w], ps[:, p:p + w],
                                mybir.ActivationFunctionType.Silu)
                        else:
                            nc.vector.tensor_mul(gh[:, mp, t:t + w],
                                                 gh[:, mp, t:t + w],
                                                 ps[:, p:p + w])
            for m in range(MD):
                pso = pspool.tile([128, NBC], dt.float32, tag="ps",
                                  name=f"pso_{n0}_{m}")
                for k in range(KF):
                    for t, p, w in slc:
                        nc.tensor.matmul(
                            pso[:, p:p + w], lhsT=wd_sb[:, k, ts(m, 128)],
                            rhs=gh[:, k, t:t + w],
                            start=(k == 0), stop=(k == KF - 1))
                o_sb = opool.tile([128, NBC], dt.float32, tag="o",
                                  name=f"o_{n0}_{m}")
                for t, p, w in slc:
                    nc.vector.tensor_copy(o_sb[:, t:t + w], pso[:, p:p + w])
                    nc.sync.dma_start(outT_r[:, m, n0 + t:n0 + t + w],
                                      o_sb[:, t:t + w])

        def body():
            for i, (n0, nw, slc) in enumerate(chunk_list()):
                do_chunk(n0, nw, slc, i == 0)

        if hw_loop:
            with tc.For_i(0, hw_loop, 1):
                body()
        else:
            body()
    nc.finalize()
    return nc


def build_nc_mmonly(C, hw_loop=0):
    """Microbench: gate-phase-like pure matmul stream (resident operands).
    Per-rep predicted 2.4GHz time: C*128/2.4e9 ns. Measures real PE rate."""
    nc = bacc.Bacc("TRN2", target_bir_lowering=False, debug=False, num_devices=E)
    dt = mybir.dt
    xT = nc.dram_tensor("xT", [D, C], dt.float16, kind="ExternalInput")
    wgu = nc.dram_tensor("wguT", [D, F2], dt.float16, kind="ExternalInput")
    outT = nc.dram_tensor("outT", [D, C], dt.float32, kind="ExternalOutput")
    NBC = 512
    with TileContext(nc) as tc, ExitStack() as ctx:
        wpool = ctx.enter_context(tc.tile_pool(name="weights", bufs=1))
        wg_sb = wpool.tile([128, KD, F2], dt.float16, tag="wg")
        xpool = ctx.enter_context(tc.tile_pool(name="x", bufs=1))
        x_sb = xpool.tile([128, KD, NBC], dt.float16, tag="x")
        gpool = ctx.enter_context(tc.tile_pool(name="g", bufs=2))
        pspool = ctx.enter_context(tc.tile_pool(name="ps", bufs=4, space="PSUM"))
        nc.sync.dma_start(x_sb[:], xT.rearrange("(k p) t -> p k t", p=128)[:, :, :NBC])
        for k in range(KD):
            nc.sync.dma_start(wg_sb[:, k], wgu.rearrange("(k p) f -> p k f", p=128)[:, k])

        def body():
            # same MM count as one full v7 rep-worth of gate+up+down per
            # 512 tokens x (C/512): 384 * ceil(C/512) MMs of N=512
            for rep in range(-(-C // NBC)):
                for mp in range(KF * 2 + MD):
                    ps = pspool.tile([128, NBC], dt.float32, tag="ps",
                                     name=f"ps_{rep}_{mp}")
                    f0 = (mp * 128) % F2
                    for k in range(KD):
                        nc.tensor.matmul(
                            ps[:], lhsT=wg_sb[:, k, ds(f0, 128)],
                            rhs=x_sb[:, k, :],
                            start=(k == 0), stop=(k == KD - 1))
                    g_sb = gpool.tile([128, NBC], dt.float32, tag="g",
                                      name=f"g_{rep}_{mp}")
                    nc.scalar.activation(g_sb[:], ps[:],
                                         mybir.ActivationFunctionType.Silu)
            nc.sync.dma_start(
                outT.rearrange("(m p) t -> p m t", p=128)[:, 0, :NBC],
                g_sb[:])

        if hw_loop:
            with tc.For_i(0, hw_loop, 1):
                body()
        else:
            body()
    nc.finalize()
    return nc


def build_nc_v8(C, hw_loop=0):
    """v7 + weight streams spread across the first chunk's phases."""
    return build_nc_v7(C, hw_loop=hw_loop, spread_weights=True)


def build_nc_v9(C, hw_loop=0):
    """v7 + bf16 output (halves out-DMA bytes) + all out-DMAs on the SP
    queue (the ACT queue then carries only gate stripes at a rep/kernel
    start, removing queue-level collision between the down-phase writeback
    and the next gate-weight stream)."""
    return build_nc_v7(C, hw_loop=hw_loop, out_bf16=True, outs_on_sp=True)


def build_nc_v7(C, hw_loop=0, weights_outside=False, spread_weights=False,
                out_bf16=False, outs_on_sp=False):
    """Near-equal 3-slice chunks + stripe-major weight tiles.

    Design notes (from TimelineSim analysis of big/v5/v6):
    - each dma_start costs ~650ns serial issue -> few, large descriptors;
    - Tile dep-tracking uses flattened-free-dim bounding boxes -> weight
      tiles are laid out stripe-major ([128, stripe, k, cols]) so one
      stripe DMA = one exact-bbox descriptor;
    - PSUM recycle (matmul group -> silu/mul -> free) takes ~2.3us, so
      chunk slices are sized so a group is >=3us: near-equal chunks of
      ~1052 tokens, 3 bank-aligned slices each, [128,1536] psum x2 bufs;
    - gate weights stream in 256-col stripes (consumption order), up half
      and down weights in 512-col stripes;
    - first chunk's first two gate groups run slice-outer so the first
      matmul needs only slice0 of x + the first gate stripe (~1 MB).
    """
    nc = bacc.Bacc("TRN2", target_bir_lowering=False, debug=False, num_devices=E)
    dt = mybir.dt
    NB = 1536
    out_dt = dt.bfloat16 if out_bf16 else dt.float32
    xT = nc.dram_tensor("xT", [D, C], dt.float16, kind="ExternalInput")
    wgu = nc.dram_tensor("wguT", [D, F2], dt.float16, kind="ExternalInput")
    wd = nc.dram_tensor("wdT", [F, D], dt.float16, kind="ExternalInput")
    outT = nc.dram_tensor("outT", [D, C], out_dt, kind="ExternalOutput")

    GS = 256   # gate-half weight stripe width
    WS = 512   # up-half / down weight stripe width
    NGS = F // GS
    with TileContext(nc) as tc, ExitStack() as ctx:
        wpool = ctx.enter_context(tc.tile_pool(name="weights", bufs=1))
        # [128, stripe, k, cols]: one DMA per stripe with an exact bbox
        wg_sb = wpool.tile([128, NGS, KD, GS], dt.float16, tag="wg")
        wu_sb = wpool.tile([128, F // WS, KD, WS], dt.float16, tag="wu")
        wd_sb = wpool.tile([128, D // WS, KF, WS], dt.float16, tag="wd")

        xpool = ctx.enter_context(tc.tile_pool(name="x", bufs=1))
        ghpool = ctx.enter_context(tc.tile_pool(name="gh", bufs=1))
        opool = ctx.enter_context(tc.tile_pool(name="o", bufs=2))
        pspool = ctx.enter_context(tc.tile_pool(name="ps", bufs=2, space="PSUM"))

        xT_r = xT.rearrange("(k p) t -> p k t", p=128)
        wgu_r = wgu.rearrange("(k p) f -> p k f", p=128)
        wd_r = wd.rearrange("(k p) m -> p k m", p=128)
        outT_r = outT.rearrange("(m p) t -> p m t", p=128)

        def gate_w(mp):  # lhsT for gate col-tile mp (128 cols)
            f0 = mp * 128
            return wg_sb[:, f0 // GS, :, (f0 % GS):(f0 % GS) + 128]

        def up_w(mp):
            f0 = mp * 128
            return wu_sb[:, f0 // WS, :, (f0 % WS):(f0 % WS) + 128]

        def down_w(m):
            f0 = m * 128
            return wd_sb[:, f0 // WS, :, (f0 % WS):(f0 % WS) + 128]

        def chunk_list():
            nch = max(1, -(-C // NB))
            base, ext = divmod(C, nch)
            sizes = [base + (1 if i < ext else 0) for i in range(nch)]
            out, n0 = [], 0
            for nw in sizes:
                ns = min(3, -(-nw // NT))
                wv, we = divmod(nw, ns)
                widths = [wv + (1 if i < we else 0) for i in range(ns)]
                slc, t = [], 0
                for i, w in enumerate(widths):
                    slc.append((t, i * NT, w))
                    t += w
                out.append((n0, nw, slc))
                n0 += nw
            return out

        def emit_gate_dmas():
            # gate stripes issue on the Activation HWDGE queue, everything
            # else on SP: the two queues issue descriptors in parallel
            # (~650ns serial issue each), so the gate path isn't starved.
            for s in range(NGS):
                nc.scalar.dma_start(wg_sb[:, s], wgu_r[:, :, s * GS:(s + 1) * GS])

        def emit_up_dmas():
            for s in range(F // WS):
                nc.sync.dma_start(wu_sb[:, s],
                                  wgu_r[:, :, F + s * WS:F + (s + 1) * WS])

        def emit_down_dmas():
            for s in range(D // WS):
                nc.sync.dma_start(wd_sb[:, s], wd_r[:, :, s * WS:(s + 1) * WS])

        def emit_weight_dmas():
            emit_gate_dmas()
            emit_up_dmas()
            emit_down_dmas()

        def do_chunk(n0, nw, slc, first_chunk, last_chunk):
            x_sb = xpool.tile([128, 3, KD, NT], dt.float16, tag="x")
            for si, (t, p, w) in enumerate(slc):
                nc.sync.dma_start(x_sb[:, si, :, :w],
                                  xT_r[:, :, n0 + t:n0 + t + w])
            if first_chunk and not weights_outside:
                if spread_weights:
                    emit_gate_dmas()
                else:
                    emit_weight_dmas()
            gh = ghpool.tile([128, KF, NB], dt.float16, tag="gh")
            for phase in (0, 1):  # 0: gate+silu, 1: up+mul-in-place
                for mp in range(KF):
                    if (first_chunk and spread_weights and not weights_outside
                            and mp == KF // 2):
                        # spread the bulk weight streams: up half midway
                        # through the gate phase, down weights midway through
                        # the up phase (~14us of lead each, > the ~11.5us of
                        # data) — decongests the rep-boundary DMA burst
                        if phase == 0:
                            emit_up_dmas()
                        else:
                            emit_down_dmas()
                    ps = pspool.tile([128, NB], dt.float32, tag="ps",
                                     name=f"ps_{n0}_{phase}_{mp}")
                    wsel = gate_w(mp) if phase == 0 else up_w(mp)
                    slice_outer = first_chunk and phase == 0 and mp < 2
                    if slice_outer:
                        loop = [(k, si) for si in range(len(slc))
                                for k in range(KD)]
                    else:
                        loop = [(k, si) for k in range(KD)
                                for si in range(len(slc))]
                    for k, si in loop:
                        t, p, w = slc[si]
                        nc.tensor.matmul(
                            ps[:, p:p + w], lhsT=wsel[:, k],
                            rhs=x_sb[:, si, k, :w],
                            start=(k == 0), stop=(k == KD - 1))
                    for t, p, w in slc:
                        if phase == 0:
                            nc.scalar.activation(
                                gh[:, mp, t:t + w], ps[:, p:p + w],
                                mybir.ActivationFunctionType.Silu)
                        else:
                            nc.vector.tensor_mul(gh[:, mp, t:t + w],
                                                 gh[:, mp, t:t + w],
                                                 ps[:, p:p + w])
            for m in range(MD):
                pso = pspool.tile([128, NB], dt.float32, tag="ps",
                                  name=f"pso_{n0}_{m}")
                # last m of the last chunk runs slice-outer: slice s's
                # accumulation closes ~5us before the kernel end, so its
                # eviction+DMA pipeline under the remaining matmuls and the
                # serial tail is one ~350-token eviction.
                tail = last_chunk and m == MD - 1
                o_sb = opool.tile([128, NB], out_dt, tag="o",
                                  name=f"o_{n0}_{m}")
                if tail:
                    for si, (t, p, w) in enumerate(slc):
                        for k in range(KF):
                            nc.tensor.matmul(
                                pso[:, p:p + w], lhsT=down_w(m)[:, k],
                                rhs=gh[:, k, t:t + w],
                                start=(k == 0), stop=(k == KF - 1))
                        nc.vector.tensor_copy(o_sb[:, t:t + w], pso[:, p:p + w])
                        dge = nc.sync if (outs_on_sp or si % 2 == 0) \
                            else nc.scalar
                        dge.dma_start(outT_r[:, m, n0 + t:n0 + t + w],
                                      o_sb[:, t:t + w])
                else:
                    for k in range(KF):
                        for t, p, w in slc:
                            nc.tensor.matmul(
                                pso[:, p:p + w], lhsT=down_w(m)[:, k],
                                rhs=gh[:, k, t:t + w],
                                start=(k == 0), stop=(k == KF - 1))
                    for t, p, w in slc:
                        nc.vector.tensor_copy(o_sb[:, t:t + w], pso[:, p:p + w])
                        dge = nc.sync if (outs_on_sp or m % 2 == 0) \
                            else nc.scalar
                        dge.dma_start(outT_r[:, m, n0 + t:n0 + t + w],
                                      o_sb[:, t:t + w])

        def body():
            chunks = chunk_list()
            for i, (n0, nw, slc) in enumerate(chunks):
                do_chunk(n0, nw, slc, i == 0, i == len(chunks) - 1)

        if weights_outside:
            emit_weight_dmas()
        if hw_loop:
            with tc.For_i(0, hw_loop, 1):
                body()
        else:
            body()
    nc.finalize()
    return nc


def build_nc_v7nw(C, hw_loop=0):
    """v7 with weight DMAs hoisted out of the hw_loop (microbench: isolates
    the per-rep 12MB weight re-stream from the loop differential)."""
    return build_nc_v7(C, hw_loop=hw_loop, weights_outside=True)


# the shipped kernel variant (used by get_nc and test.py's timing loop)
BUILD = build_nc_v9


def route(x, expert_indices):
    """Sort tokens by expert; return (order, counts, capacity C)."""
    idx = np.asarray(expert_indices)
    order = np.argsort(idx, kind="stable")
    counts = np.bincount(idx, minlength=E).astype(np.int64)
    C = max(NT, int(-(-counts.max() // 8) * 8))
    return order, counts, C


def make_in_maps(x, expert_indices, gate_up_weight, down_weight):
    order, counts, C = route(x, expert_indices)
    x_sorted = np.asarray(x, dtype=np.float32)[order]
    offs = np.zeros(E + 1, dtype=np.int64)
    np.cumsum(counts, out=offs[1:])
    wguT = np.ascontiguousarray(
        np.transpose(np.asarray(gate_up_weight), (0, 2, 1))).astype(F16)
    wdT = np.ascontiguousarray(
        np.transpose(np.asarray(down_weight), (0, 2, 1))).astype(F16)
    in_maps = []
    for e in range(E):
        xe = np.zeros((C, D), dtype=np.float32)
        xe[: counts[e]] = x_sorted[offs[e]: offs[e + 1]]
        in_maps.append({
            "xT": np.ascontiguousarray(xe.T).astype(F16),
            "wguT": wguT[e],
            "wdT": wdT[e],
        })
    return in_maps, order, counts, C


def assemble_output(results, order, counts):
    T = int(counts.sum())
    out = np.empty((T, D), dtype=np.float32)
    offs = np.zeros(E + 1, dtype=np.int64)
    np.cumsum(counts, out=offs[1:])
    sorted_out = np.empty((T, D), dtype=np.float32)
    for e in range(E):
        sorted_out[offs[e]: offs[e + 1]] = results[e]["outT"].T[: counts[e]]
    out[order] = sorted_out
    return out


def kernel(x, expert_indices, gate_up_weight, down_weight):
    in_maps, order, counts, C = make_in_maps(
        x, expert_indices, gate_up_weight, down_weight)
    nc = get_nc(C)
    res = run_bass_kernel_spmd(nc, in_maps, core_ids=list(range(E)))
    return assemble_output(res.results, order, counts)



# revision 35
# speedup vs baseline: 1.0116x; 1.0095x over previous
"""MoE batched-experts kernel for Trainium2 (8 NeuronCores, expert-parallel).

Contract: kernel(**inputs) takes the FULL unsharded inputs
  x:              [T, D]      float32   (T=16384, D=1024)
  expert_indices: [T]         int32     (values in [0, 8))
  gate_up_weight: [E, 2F, D]  float32   (E=8, F=2048)
  down_weight:    [E, D, F]   float32
and returns the FULL output [T, D] float32:
  per token t with expert e: h = silu(x @ gu[e,:F].T) * (x @ gu[e,F:].T);
  out = h @ dw[e].T

Strategy: expert-parallel. The host routes (sorts) tokens by expert, pads
each expert's group to a common capacity C (max count rounded up to 8), and
core e runs a dense FFN for expert e on its token group. All operands are
pre-transposed / pre-cast to fp16 on the host (same PE rate as bf16, 8x the
mantissa: rel-err ~6e-4 vs ~4.5e-3) so the device kernel is pure matmul +
silu*mul with no on-chip transposes:
  core e computes outT = w_d @ (silu(w_gT.T @ xT) * (w_uT.T @ xT))
with xT [D, C], producing outT [D, C] (bf16 in the shipped build_nc_v9:
halves writeback DMA, rel-err 3.4e-3 vs the 2e-2 gate); the host upcasts,
transposes back and unpermutes.
"""

import numpy as np
import ml_dtypes

import concourse.bass as bass
import concourse.mybir as mybir
from concourse import bacc
from concourse.tile import TileContext
from concourse.bass import ts, ds
from concourse.bass_utils import run_bass_kernel_spmd
from contextlib import ExitStack

BF16 = ml_dtypes.bfloat16
F16 = np.float16  # operand dtype for matmuls: same PE rate as bf16, 8x mantissa

D = 1024      # d_model
F = 2048      # d_ff
F2 = 2 * F    # gate+up
E = 8         # experts == cores
KD = D // 128   # 8  k-tiles over d_model
KF = F // 128   # 16 k-tiles over d_ff
MD = D // 128   # 8  m-tiles over d_model (output)
NT = 512        # token chunk (one PSUM bank at fp32)

_nc_cache = {}


def build_nc(C, repeats=1, hw_loop=0):
    """Build the per-core dense-FFN Bass program for token capacity C.

    repeats>1 re-emits the whole compute body (unrolled); hw_loop>0 wraps the
    body in a hardware For_i loop. Both are timing aids: slope of time vs
    repetition count isolates true exec time from dispatch overhead."""
    nc = bacc.Bacc("TRN2", target_bir_lowering=False, debug=False, num_devices=E)
    dt = mybir.dt
    xT = nc.dram_tensor("xT", [D, C], dt.bfloat16, kind="ExternalInput")
    wgu = nc.dram_tensor("wguT", [D, F2], dt.bfloat16, kind="ExternalInput")
    wd = nc.dram_tensor("wdT", [F, D], dt.bfloat16, kind="ExternalInput")
    outT = nc.dram_tensor("outT", [D, C], dt.float32, kind="ExternalOutput")

    with TileContext(nc) as tc, ExitStack() as ctx:
        wpool = ctx.enter_context(tc.tile_pool(name="weights", bufs=1))
        wgu_sb = wpool.tile([128, KD, F2], dt.bfloat16, tag="wgu")
        nc.sync.dma_start(wgu_sb[:], wgu.rearrange("(k p) f -> p k f", p=128))
        wd_sb = wpool.tile([128, KF, D], dt.bfloat16, tag="wd")
        nc.sync.dma_start(wd_sb[:], wd.rearrange("(k p) f -> p k f", p=128))

        xpool = ctx.enter_context(tc.tile_pool(name="x", bufs=2))
        hpool = ctx.enter_context(tc.tile_pool(name="h", bufs=2))
        spool = ctx.enter_context(tc.tile_pool(name="silu", bufs=4))
        opool = ctx.enter_context(tc.tile_pool(name="o", bufs=4))
        pg = ctx.enter_context(tc.tile_pool(name="pg", bufs=2, space="PSUM"))
        pu = ctx.enter_context(tc.tile_pool(name="pu", bufs=2, space="PSUM"))
        po = ctx.enter_context(tc.tile_pool(name="po", bufs=2, space="PSUM"))

        xT_r = xT.rearrange("(k p) t -> p k t", p=128)
        outT_r = outT.rearrange("(m p) t -> p m t", p=128)

        def body():
            for n0 in [i for _ in range(repeats) for i in range(0, C, NT)]:
                nt = min(NT, C - n0)
                x_sb = xpool.tile([128, KD, NT], dt.bfloat16, tag="x")
                nc.sync.dma_start(x_sb[:, :, :nt], xT_r[:, :, n0:n0 + nt])
                h_sb = hpool.tile([128, KF, NT], dt.bfloat16, tag="h")
                for mp in range(KF):
                    psg = pg.tile([128, NT], dt.float32, tag="pg")
                    for k in range(KD):
                        nc.tensor.matmul(
                            psg[:, :nt], lhsT=wgu_sb[:, k, ts(mp, 128)],
                            rhs=x_sb[:, k, :nt], start=(k == 0), stop=(k == KD - 1))
                    psu = pu.tile([128, NT], dt.float32, tag="pu")
                    for k in range(KD):
                        nc.tensor.matmul(
                            psu[:, :nt], lhsT=wgu_sb[:, k, ds(F + mp * 128, 128)],
                            rhs=x_sb[:, k, :nt], start=(k == 0), stop=(k == KD - 1))
                    sil = spool.tile([128, NT], dt.bfloat16, tag="sil")
                    nc.scalar.activation(sil[:, :nt], psg[:, :nt],
                                         mybir.ActivationFunctionType.Silu)
                    nc.vector.tensor_mul(h_sb[:, mp, :nt], sil[:, :nt], psu[:, :nt])
                for m in range(MD):
                    pso = po.tile([128, NT], dt.float32, tag="po")
                    for k in range(KF):
                        nc.tensor.matmul(
                            pso[:, :nt], lhsT=wd_sb[:, k, ts(m, 128)],
                            rhs=h_sb[:, k, :nt], start=(k == 0), stop=(k == KF - 1))
                    o_sb = opool.tile([128, NT], dt.float32, tag="o")
                    nc.vector.tensor_copy(o_sb[:, :nt], pso[:, :nt])
                    nc.sync.dma_start(outT_r[:, m, n0:n0 + nt], o_sb[:, :nt])

        if hw_loop:
            with tc.For_i(0, hw_loop, 1):
                body()
        else:
            body()
    nc.finalize()
    return nc


def build_nc_wide(C, hw_loop=0):
    """Variant: 1024-token compute chunks with [128,1024] PSUM tiles.

    - halves ACT/DVE eviction instruction count (wide silu/mul)
    - consecutive matmuls share the same lhsT (LDW dedup opportunity)
    - PSUM banks: pg 2x2 + pu 1x2 + po 2x1 = 8
    """
    nc = bacc.Bacc("TRN2", target_bir_lowering=False, debug=False, num_devices=E)
    dt = mybir.dt
    NW = 1024
    xT = nc.dram_tensor("xT", [D, C], dt.bfloat16, kind="ExternalInput")
    wgu = nc.dram_tensor("wguT", [D, F2], dt.bfloat16, kind="ExternalInput")
    wd = nc.dram_tensor("wdT", [F, D], dt.bfloat16, kind="ExternalInput")
    outT = nc.dram_tensor("outT", [D, C], dt.float32, kind="ExternalOutput")

    with TileContext(nc) as tc, ExitStack() as ctx:
        wpool = ctx.enter_context(tc.tile_pool(name="weights", bufs=1))
        wgu_sb = wpool.tile([128, KD, F2], dt.bfloat16, tag="wgu")
        nc.sync.dma_start(wgu_sb[:], wgu.rearrange("(k p) f -> p k f", p=128))
        wd_sb = wpool.tile([128, KF, D], dt.bfloat16, tag="wd")
        nc.sync.dma_start(wd_sb[:], wd.rearrange("(k p) f -> p k f", p=128))

        xpool = ctx.enter_context(tc.tile_pool(name="x", bufs=1))
        hpool = ctx.enter_context(tc.tile_pool(name="h", bufs=3))
        spool = ctx.enter_context(tc.tile_pool(name="silu", bufs=3))
        opool = ctx.enter_context(tc.tile_pool(name="o", bufs=4))
        pg = ctx.enter_context(tc.tile_pool(name="pg", bufs=2, space="PSUM"))
        pu = ctx.enter_context(tc.tile_pool(name="pu", bufs=1, space="PSUM"))
        po = ctx.enter_context(tc.tile_pool(name="po", bufs=2, space="PSUM"))

        xT_r = xT.rearrange("(k p) t -> p k t", p=128)
        outT_r = outT.rearrange("(m p) t -> p m t", p=128)

        def do_chunk(n0, nw):
            # nw tokens starting at n0; nw in {1024, C % 1024}
            nh = (nw + NT - 1) // NT  # h sub-chunks of <=512
            x_sb = xpool.tile([128, KD, NW], dt.bfloat16, tag="x")
            nc.sync.dma_start(x_sb[:, :, :nw], xT_r[:, :, n0:n0 + nw])
            h_sbs = [hpool.tile([128, KF, NT], dt.bfloat16, tag="h",
                                name=f"h_{n0}_{s}")
                     for s in range(nh)]
            for mp in range(KF):
                psg = pg.tile([128, NW], dt.float32, tag="pg")
                for k in range(KD):
                    for s in range(nh):
                        w = min(NT, nw - s * NT)
                        nc.tensor.matmul(
                            psg[:, s * NT:s * NT + w],
                            lhsT=wgu_sb[:, k, ts(mp, 128)],
                            rhs=x_sb[:, k, s * NT:s * NT + w],
                            start=(k == 0), stop=(k == KD - 1))
                psu = pu.tile([128, NW], dt.float32, tag="pu")
                for k in range(KD):
                    for s in range(nh):
                        w = min(NT, nw - s * NT)
                        nc.tensor.matmul(
                            psu[:, s * NT:s * NT + w],
                            lhsT=wgu_sb[:, k, ds(F + mp * 128, 128)],
                            rhs=x_sb[:, k, s * NT:s * NT + w],
                            start=(k == 0), stop=(k == KD - 1))
                sil = spool.tile([128, NW], dt.bfloat16, tag="sil")
                nc.scalar.activation(sil[:, :nw], psg[:, :nw],
                                     mybir.ActivationFunctionType.Silu)
                for s in range(nh):
                    w = min(NT, nw - s * NT)
                    nc.vector.tensor_mul(h_sbs[s][:, mp, :w],
                                         sil[:, s * NT:s * NT + w],
                                         psu[:, s * NT:s * NT + w])
            for m in range(MD):
                for s in range(nh):
                    w = min(NT, nw - s * NT)
                    pso = po.tile([128, NT], dt.float32, tag="po")
                    for k in range(KF):
                        nc.tensor.matmul(
                            pso[:, :w], lhsT=wd_sb[:, k, ts(m, 128)],
                            rhs=h_sbs[s][:, k, :w],
                            start=(k == 0), stop=(k == KF - 1))
                    o_sb = opool.tile([128, NT], dt.float32, tag="o")
                    nc.vector.tensor_copy(o_sb[:, :w], pso[:, :w])
                    nc.sync.dma_start(outT_r[:, m, n0 + s * NT:n0 + s * NT + w],
                                      o_sb[:, :w])

        def body():
            for n0 in range(0, C, NW):
                do_chunk(n0, min(NW, C - n0))

        if hw_loop:
            with tc.For_i(0, hw_loop, 1):
                body()
        else:
            body()
    nc.finalize()
    return nc


def get_nc(C):
    # build_nc_v7: TimelineSim 355.9us vs build_nc_big's 365.3us; fewer DMA
    # descriptors (~26 vs 184; each costs ~650ns of serial HWDGE issue) and
    # 1:3 LDW:MM in the down phase (vs 1:1).  HW loop-differential measures
    # ~433-447us/rep — at the machine's sustained matmul-stream limit: a
    # pure-MM microbench with zero DMA/deps measures 426us for the same
    # column count (PE effectively ~1.9GHz under sustained load, not 2.4).
    # fp8 DoubleRow (1.44x PE) was evaluated and rejected: e4m3 operands
    # give rel-err ~7e-2 on this problem vs the 2e-2 gate (measured in
    # numpy emulation; error is mantissa-limited, scaling cannot fix it).
    if C not in _nc_cache:
        _nc_cache[C] = BUILD(C)
    return _nc_cache[C]


def build_nc_big(C, hw_loop=0):
    """Variant: 1536-token chunks ([128,1536] 3-bank PSUM tiles).

    Streams 3x512 tokens per weight load (LDW count 1920 -> ~768), evicts
    gate via silu into a chunk-resident SBUF tensor, then multiplies the up
    projection into it in place. PSUM: pp 2x3 + po 2x1 = 8 banks.
    """
    nc = bacc.Bacc("TRN2", target_bir_lowering=False, debug=False, num_devices=E)
    dt = mybir.dt
    NB = 1536
    xT = nc.dram_tensor("xT", [D, C], dt.float16, kind="ExternalInput")
    wgu = nc.dram_tensor("wguT", [D, F2], dt.float16, kind="ExternalInput")
    wd = nc.dram_tensor("wdT", [F, D], dt.float16, kind="ExternalInput")
    outT = nc.dram_tensor("outT", [D, C], dt.float32, kind="ExternalOutput")

    with TileContext(nc) as tc, ExitStack() as ctx:
        # per-k weight tiles with separate DMAs; the first chunk's x tiles
        # are DMA'd BEFORE the weights (see do_chunk) so the PE's first
        # matmul group is gated on ~4 MB, not the full 16 MB input set.
        wpool = ctx.enter_context(tc.tile_pool(name="weights", bufs=1))
        wgu_k = [wpool.tile([128, F2], dt.float16, tag=f"wgu{k}",
                            name=f"wgu{k}") for k in range(KD)]
        wd_k = [wpool.tile([128, D], dt.float16, tag=f"wd{k}",
                           name=f"wd{k}") for k in range(KF)]

        xpool = ctx.enter_context(tc.tile_pool(name="x", bufs=1))
        ghpool = ctx.enter_context(tc.tile_pool(name="gh", bufs=1))
        opool = ctx.enter_context(tc.tile_pool(name="o", bufs=4))
        pp = ctx.enter_context(tc.tile_pool(name="pp", bufs=2, space="PSUM"))
        po = ctx.enter_context(tc.tile_pool(name="po", bufs=2, space="PSUM"))

        xT_r = xT.rearrange("(k p) t -> p k t", p=128)
        outT_r = outT.rearrange("(m p) t -> p m t", p=128)

        def slices(nw):
            return [(s, min(NT, nw - s)) for s in range(0, nw, NT)]

        def do_chunk(n0, nw):
            x_sb = xpool.tile([128, KD, NB], dt.float16, tag="x")
            if first[0]:
                first[0] = False
                # interleave x and gate/up weight k-tiles so the first
                # matmul group's operands stream in consumption order
                for k in range(KD):
                    nc.sync.dma_start(x_sb[:, k, :nw], xT_r[:, k, n0:n0 + nw])
                    # first 512 f-columns land first so the k-th LDW of the
                    # first gate group unblocks after ~0.4 MB, not 1 MB
                    nc.sync.dma_start(wgu_k[k][:, :NT],
                                      wgu[k * 128:(k + 1) * 128, :NT])
                    nc.sync.dma_start(wgu_k[k][:, NT:],
                                      wgu[k * 128:(k + 1) * 128, NT:])
                for k in range(KF):
                    nc.sync.dma_start(wd_k[k][:], wd[k * 128:(k + 1) * 128, :])
            else:
                for k in range(KD):
                    nc.sync.dma_start(x_sb[:, k, :nw], xT_r[:, k, n0:n0 + nw])
            gh = ghpool.tile([128, KF, NB], dt.float16, tag="gh")
            for phase in (0, 1):  # 0: gate+silu, 1: up+mul-in-place
                for mp in range(KF):
                    ps = pp.tile([128, NB], dt.float32, tag="pp",
                                 name=f"ps_{n0}_{phase}_{mp}")
                    f0 = mp * 128 if phase == 0 else F + mp * 128
                    for k in range(KD):
                        for s, w in slices(nw):
                            nc.tensor.matmul(
                                ps[:, s:s + w],
                                lhsT=wgu_k[k][:, ds(f0, 128)],
                                rhs=x_sb[:, k, s:s + w],
                                start=(k == 0), stop=(k == KD - 1))
                    if phase == 0:
                        nc.scalar.activation(gh[:, mp, :nw], ps[:, :nw],
                                             mybir.ActivationFunctionType.Silu)
                    else:
                        nc.vector.tensor_mul(gh[:, mp, :nw], gh[:, mp, :nw],
                                             ps[:, :nw])
            for m in range(MD):
                for s, w in slices(nw):
                    pso = po.tile([128, NT], dt.float32, tag="po",
                                  name=f"pso_{n0}_{m}_{s}")
                    for k in range(KF):
                        nc.tensor.matmul(
                            pso[:, :w], lhsT=wd_k[k][:, ts(m, 128)],
                            rhs=gh[:, k, s:s + w],
                            start=(k == 0), stop=(k == KF - 1))
                    o_sb = opool.tile([128, NT], dt.float32, tag="o",
                                      name=f"o_{n0}_{m}_{s}")
                    nc.vector.tensor_copy(o_sb[:, :w], pso[:, :w])
                    nc.sync.dma_start(outT_r[:, m, n0 + s:n0 + s + w],
                                      o_sb[:, :w])

        first = [True]

        def body():
            # smallest chunk first: the cold-start stall is gated on the
            # first chunk's x DMA, so lead with the cheapest one
            chunks = [(n0, min(NB, C - n0)) for n0 in range(0, C, NB)]
            chunks.sort(key=lambda c: c[1])
            for n0, nw in chunks:
                do_chunk(n0, nw)



        if hw_loop:
            with tc.For_i(0, hw_loop, 1):
                body()
        else:
            body()
    nc.finalize()
    return nc


def build_nc_v3(C, hw_loop=0):
    """Tuned variant of build_nc_big:

    - weight DMAs striped in consumption order (512-col stripes across all
      k-tiles, gate half first, then up half, then wd) so the first gate
      phase is never DMA-starved;
    - remainder-chunk token slices equalized (e.g. 568 -> 284+284, not
      512+56) so no runt matmuls pay the per-MM issue floor;
    - down phase restructured m -> k -> s with a full-width [128, NB] PSUM
      tile from the shared pool: 1 LDW per 3 matmuls instead of 1:1.
    PSUM: ps pool 2x3 banks = 6 of 8 banks.
    """
    nc = bacc.Bacc("TRN2", target_bir_lowering=False, debug=False, num_devices=E)
    dt = mybir.dt
    NB = 1536
    xT = nc.dram_tensor("xT", [D, C], dt.float16, kind="ExternalInput")
    wgu = nc.dram_tensor("wguT", [D, F2], dt.float16, kind="ExternalInput")
    wd = nc.dram_tensor("wdT", [F, D], dt.float16, kind="ExternalInput")
    outT = nc.dram_tensor("outT", [D, C], dt.float32, kind="ExternalOutput")

    with TileContext(nc) as tc, ExitStack() as ctx:
        wpool = ctx.enter_context(tc.tile_pool(name="weights", bufs=1))
        wgu_k = [wpool.tile([128, F2], dt.float16, tag=f"wgu{k}",
                            name=f"wgu{k}") for k in range(KD)]
        wd_k = [wpool.tile([128, D], dt.float16, tag=f"wd{k}",
                           name=f"wd{k}") for k in range(KF)]

        xpool = ctx.enter_context(tc.tile_pool(name="x", bufs=1))
        ghpool = ctx.enter_context(tc.tile_pool(name="gh", bufs=1))
        opool = ctx.enter_context(tc.tile_pool(name="o", bufs=2))
        pspool = ctx.enter_context(tc.tile_pool(name="ps", bufs=2, space="PSUM"))

        xT_r = xT.rearrange("(k p) t -> p k t", p=128)
        outT_r = outT.rearrange("(m p) t -> p m t", p=128)

        def slices(nw):
            ns = (nw + NT - 1) // NT
            w = -(-nw // ns)  # equal widths, last may be smaller by <ns
            return [(s, min(w, nw - s)) for s in range(0, nw, w)]

        def chunk_list():
            chunks = []
            rem = C
            while rem > 0:
                take = NB if rem >= NB else rem
                chunks.append(take)
                rem -= take
            sizes = sorted(chunks)  # smallest first: cheapest cold start
            offs = []
            n0 = 0
            for s in sizes:
                offs.append((n0, s))
                n0 += s
            return offs

        def emit_weight_dmas():
            # gate half, then up half: 512-col stripes across all k-tiles in
            # the order the first gate phase consumes them
            for half in (0, F):
                for c0 in range(half, half + F, 512):
                    for k in range(KD):
                        nc.sync.dma_start(wgu_k[k][:, c0:c0 + 512],
                                          wgu[k * 128:(k + 1) * 128, c0:c0 + 512])
            for c0 in range(0, D, 512):
                for k in range(KF):
                    nc.sync.dma_start(wd_k[k][:, c0:c0 + 512],
                                      wd[k * 128:(k + 1) * 128, c0:c0 + 512])

        def do_chunk(n0, nw, first_chunk):
            x_sb = xpool.tile([128, KD, NB], dt.float16, tag="x")
            if first_chunk:
                for k in range(KD):
                    nc.sync.dma_start(x_sb[:, k, :nw], xT_r[:, k, n0:n0 + nw])
                emit_weight_dmas()
            else:
                for k in range(KD):
                    nc.sync.dma_start(x_sb[:, k, :nw], xT_r[:, k, n0:n0 + nw])
            gh = ghpool.tile([128, KF, NB], dt.float16, tag="gh")
            for phase in (0, 1):  # 0: gate+silu, 1: up+mul-in-place
                for mp in range(KF):
                    ps = pspool.tile([128, NB], dt.float32, tag="ps",
                                     name=f"ps_{n0}_{phase}_{mp}")
                    f0 = mp * 128 if phase == 0 else F + mp * 128
                    for k in range(KD):
                        for s, w in slices(nw):
                            nc.tensor.matmul(
                                ps[:, s:s + w],
                                lhsT=wgu_k[k][:, ds(f0, 128)],
                                rhs=x_sb[:, k, s:s + w],
                                start=(k == 0), stop=(k == KD - 1))
                    if phase == 0:
                        nc.scalar.activation(gh[:, mp, :nw], ps[:, :nw],
                                             mybir.ActivationFunctionType.Silu)
                    else:
                        nc.vector.tensor_mul(gh[:, mp, :nw], gh[:, mp, :nw],
                                             ps[:, :nw])
            for m in range(MD):
                pso = pspool.tile([128, NB], dt.float32, tag="ps",
                                  name=f"pso_{n0}_{m}")
                for k in range(KF):
                    for s, w in slices(nw):
                        nc.tensor.matmul(
                            pso[:, s:s + w], lhsT=wd_k[k][:, ts(m, 128)],
                            rhs=gh[:, k, s:s + w],
                            start=(k == 0), stop=(k == KF - 1))
                o_sb = opool.tile([128, NB], dt.float32, tag="o",
                                  name=f"o_{n0}_{m}")
                nc.vector.tensor_copy(o_sb[:, :nw], pso[:, :nw])
                nc.sync.dma_start(outT_r[:, m, n0:n0 + nw], o_sb[:, :nw])

        def body():
            for i, (n0, nw) in enumerate(chunk_list()):
                do_chunk(n0, nw, i == 0)

        if hw_loop:
            with tc.For_i(0, hw_loop, 1):
                body()
        else:
            body()
    nc.finalize()
    return nc


def build_nc_v4(C, hw_loop=0):
    """v3 + better chunking and cold-start:

    - chunks [first(1 slice), 1536*q (3x512 slices), last(1 slice)]: the
      first/last remainder chunks absorb C%512 as two ~equal >=256-token
      1-slice chunks, so every matmul is >=256 wide (no runt-MM issue-floor
      waste), every PSUM write is bank-aligned, and both the cold-start x
      DMA and the serial eviction tail after the last matmul are small;
    - the first 512 gate-weight columns stream in 128-col pieces so the
      first matmul group unblocks after ~0.3 MB.
    """
    nc = bacc.Bacc("TRN2", target_bir_lowering=False, debug=False, num_devices=E)
    dt = mybir.dt
    NB = 1536
    xT = nc.dram_tensor("xT", [D, C], dt.float16, kind="ExternalInput")
    wgu = nc.dram_tensor("wguT", [D, F2], dt.float16, kind="ExternalInput")
    wd = nc.dram_tensor("wdT", [F, D], dt.float16, kind="ExternalInput")
    outT = nc.dram_tensor("outT", [D, C], dt.float32, kind="ExternalOutput")

    with TileContext(nc) as tc, ExitStack() as ctx:
        wpool = ctx.enter_context(tc.tile_pool(name="weights", bufs=1))
        wgu_k = [wpool.tile([128, F2], dt.float16, tag=f"wgu{k}",
                            name=f"wgu{k}") for k in range(KD)]
        wd_k = [wpool.tile([128, D], dt.float16, tag=f"wd{k}",
                           name=f"wd{k}") for k in range(KF)]

        xpool = ctx.enter_context(tc.tile_pool(name="x", bufs=1))
        ghpool = ctx.enter_context(tc.tile_pool(name="gh", bufs=1))
        opool = ctx.enter_context(tc.tile_pool(name="o", bufs=2))
        pspool = ctx.enter_context(tc.tile_pool(name="ps", bufs=2, space="PSUM"))

        xT_r = xT.rearrange("(k p) t -> p k t", p=128)
        outT_r = outT.rearrange("(m p) t -> p m t", p=128)

        def chunk_list():
            """[(n0, nw, [slice widths])]; each chunk is 1 slice of any
            width, or all-512 slices (bank alignment for free)."""
            ns = -(-C // NT)
            if ns <= 3:
                w = -(-C // ns)
                widths = [min(w, C - i * w) for i in range(ns)]
                groups = [[wi] for wi in widths]
            else:
                slack = NT * ns - C
                wf = NT - (slack + 1) // 2
                wl = NT - slack // 2
                mid = [NT] * (ns - 2)
                groups = [[wf]] + [mid[i:i + 3] for i in range(0, len(mid), 3)] \
                    + [[wl]]
            out, n0 = [], 0
            for g in groups:
                out.append((n0, sum(g), g))
                n0 += sum(g)
            return out

        def emit_weight_dmas():
            for c0 in range(0, 512, 128):  # first gate stripe: fine-grained
                for k in range(KD):
                    nc.sync.dma_start(wgu_k[k][:, c0:c0 + 128],
                                      wgu[k * 128:(k + 1) * 128, c0:c0 + 128])
            for half in (0, F):
                for c0 in range(half, half + F, 512):
                    if c0 == 0:
                        continue  # already emitted fine-grained
                    for k in range(KD):
                        nc.sync.dma_start(wgu_k[k][:, c0:c0 + 512],
                                          wgu[k * 128:(k + 1) * 128, c0:c0 + 512])
            for c0 in range(0, D, 512):
                for k in range(KF):
                    nc.sync.dma_start(wd_k[k][:, c0:c0 + 512],
                                      wd[k * 128:(k + 1) * 128, c0:c0 + 512])

        def do_chunk(n0, nw, widths, first_chunk):
            x_sb = xpool.tile([128, KD, NB], dt.float16, tag="x")
            for k in range(KD):
                nc.sync.dma_start(x_sb[:, k, :nw], xT_r[:, k, n0:n0 + nw])
            if first_chunk:
                emit_weight_dmas()
            slc = []
            s = 0
            for w in widths:
                slc.append((s, w))
                s += w
            gh = ghpool.tile([128, KF, NB], dt.float16, tag="gh")
            for phase in (0, 1):  # 0: gate+silu, 1: up+mul-in-place
                for mp in range(KF):
                    ps = pspool.tile([128, NB], dt.float32, tag="ps",
                                     name=f"ps_{n0}_{phase}_{mp}")
                    f0 = mp * 128 if phase == 0 else F + mp * 128
                    for k in range(KD):
                        for s, w in slc:
                            nc.tensor.matmul(
                                ps[:, s:s + w],
                                lhsT=wgu_k[k][:, ds(f0, 128)],
                                rhs=x_sb[:, k, s:s + w],
                                start=(k == 0), stop=(k == KD - 1))
                    if phase == 0:
                        nc.scalar.activation(gh[:, mp, :nw], ps[:, :nw],
                                             mybir.ActivationFunctionType.Silu)
                    else:
                        nc.vector.tensor_mul(gh[:, mp, :nw], gh[:, mp, :nw],
                                             ps[:, :nw])
            for m in range(MD):
                pso = pspool.tile([128, NB], dt.float32, tag="ps",
                                  name=f"pso_{n0}_{m}")
                for k in range(KF):
                    for s, w in slc:
                        nc.tensor.matmul(
                            pso[:, s:s + w], lhsT=wd_k[k][:, ts(m, 128)],
                            rhs=gh[:, k, s:s + w],
                            start=(k == 0), stop=(k == KF - 1))
                o_sb = opool.tile([128, NB], dt.float32, tag="o",
                                  name=f"o_{n0}_{m}")
                nc.vector.tensor_copy(o_sb[:, :nw], pso[:, :nw])
                nc.sync.dma_start(outT_r[:, m, n0:n0 + nw], o_sb[:, :nw])

        def body():
            for i, (n0, nw, widths) in enumerate(chunk_list()):
                do_chunk(n0, nw, widths, i == 0)

        if hw_loop:
            with tc.For_i(0, hw_loop, 1):
                body()
        else:
            body()
    nc.finalize()
    return nc


def build_nc_v5(C, hw_loop=0):
    """Bank-safe tuned variant (the successor of build_nc_big):

    - chunks [rem (2 equal slices), 1536 (3x512)...]: every matmul is
      >=256 tokens wide and every PSUM write sits in its own bank-aligned
      512-column slot (slice i of a chunk lives at psum column 512*i);
    - silu/mul/eviction/out-DMA run per-slice, so the serial tail after the
      very last matmul is one 512-wide eviction, not a whole 1536 chunk;
    - weight DMAs stream in consumption order (gate stripes across k first,
      the first stripe in 128-col pieces, then up half, then down weights);
    - down phase is m -> k -> slice with a full-width PSUM tile: one
      weight load per 3 matmuls.
    PSUM: shared ps pool 2x3 banks = 6 of 8 banks.
    """
    nc = bacc.Bacc("TRN2", target_bir_lowering=False, debug=False, num_devices=E)
    dt = mybir.dt
    NB = 1536
    xT = nc.dram_tensor("xT", [D, C], dt.float16, kind="ExternalInput")
    wgu = nc.dram_tensor("wguT", [D, F2], dt.float16, kind="ExternalInput")
    wd = nc.dram_tensor("wdT", [F, D], dt.float16, kind="ExternalInput")
    outT = nc.dram_tensor("outT", [D, C], dt.float32, kind="ExternalOutput")

    with TileContext(nc) as tc, ExitStack() as ctx:
        wpool = ctx.enter_context(tc.tile_pool(name="weights", bufs=1))
        wgu_k = [wpool.tile([128, F2], dt.float16, tag=f"wgu{k}",
                            name=f"wgu{k}") for k in range(KD)]
        wd_k = [wpool.tile([128, D], dt.float16, tag=f"wd{k}",
                           name=f"wd{k}") for k in range(KF)]

        xpool = ctx.enter_context(tc.tile_pool(name="x", bufs=1))
        ghpool = ctx.enter_context(tc.tile_pool(name="gh", bufs=1))
        opool = ctx.enter_context(tc.tile_pool(name="o", bufs=2))
        pspool = ctx.enter_context(tc.tile_pool(name="ps", bufs=2, space="PSUM"))

        xT_r = xT.rearrange("(k p) t -> p k t", p=128)
        outT_r = outT.rearrange("(m p) t -> p m t", p=128)

        def chunk_list():
            """[(n0, nw, [(tok_off, psum_off, w), ...])] — remainder first
            (2 equal slices), then full 1536 chunks (3x512)."""
            rem = C % NB
            chunks = []
            if rem:
                if rem <= NT:
                    w0 = (rem + 1) // 2
                    widths = [w0, rem - w0] if rem - w0 else [w0]
                elif rem <= 2 * NT:
                    w0 = (rem + 1) // 2
                    widths = [w0, rem - w0]
                else:
                    w0 = (rem + 2) // 3
                    widths = [w0, w0, rem - 2 * w0]
                chunks.append(widths)
            chunks += [[NT, NT, NT]] * (C // NB)
            out, n0 = [], 0
            for widths in chunks:
                slc, t = [], 0
                for i, w in enumerate(widths):
                    slc.append((t, i * NT, w))
                    t += w
                out.append((n0, sum(widths), slc))
                n0 += sum(widths)
            return out

        def emit_weight_dmas():
            for c0 in range(0, 512, 128):  # first gate stripe: fine-grained
                for k in range(KD):
                    nc.sync.dma_start(wgu_k[k][:, c0:c0 + 128],
                                      wgu[k * 128:(k + 1) * 128, c0:c0 + 128])
            for half in (0, F):
                for c0 in range(half, half + F, 512):
                    if c0 == 0:
                        continue  # emitted fine-grained above
                    for k in range(KD):
                        nc.sync.dma_start(wgu_k[k][:, c0:c0 + 512],
                                          wgu[k * 128:(k + 1) * 128, c0:c0 + 512])
            for c0 in range(0, D, 512):
                for k in range(KF):
                    nc.sync.dma_start(wd_k[k][:, c0:c0 + 512],
                                      wd[k * 128:(k + 1) * 128, c0:c0 + 512])

        def do_chunk(n0, nw, slc, first_chunk):
            x_sb = xpool.tile([128, KD, NB], dt.float16, tag="x")
            for k in range(KD):
                nc.sync.dma_start(x_sb[:, k, :nw], xT_r[:, k, n0:n0 + nw])
            if first_chunk:
                emit_weight_dmas()
            gh = ghpool.tile([128, KF, NB], dt.float16, tag="gh")
            for phase in (0, 1):  # 0: gate+silu, 1: up+mul-in-place
                for mp in range(KF):
                    ps = pspool.tile([128, NB], dt.float32, tag="ps",
                                     name=f"ps_{n0}_{phase}_{mp}")
                    f0 = mp * 128 if phase == 0 else F + mp * 128
                    for k in range(KD):
                        for t, p, w in slc:
                            nc.tensor.matmul(
                                ps[:, p:p + w],
                                lhsT=wgu_k[k][:, ds(f0, 128)],
                                rhs=x_sb[:, k, t:t + w],
                                start=(k == 0), stop=(k == KD - 1))
                    for t, p, w in slc:
                        if phase == 0:
                            nc.scalar.activation(
                                gh[:, mp, t:t + w], ps[:, p:p + w],
                                mybir.ActivationFunctionType.Silu)
                        else:
                            nc.vector.tensor_mul(gh[:, mp, t:t + w],
                                                 gh[:, mp, t:t + w],
                                                 ps[:, p:p + w])
            for m in range(MD):
                pso = pspool.tile([128, NB], dt.float32, tag="ps",
                                  name=f"pso_{n0}_{m}")
                for k in range(KF):
                    for t, p, w in slc:
                        nc.tensor.matmul(
                            pso[:, p:p + w], lhsT=wd_k[k][:, ts(m, 128)],
                            rhs=gh[:, k, t:t + w],
                            start=(k == 0), stop=(k == KF - 1))
                o_sb = opool.tile([128, NB], dt.float32, tag="o",
                                  name=f"o_{n0}_{m}")
                for t, p, w in slc:
                    nc.vector.tensor_copy(o_sb[:, t:t + w], pso[:, p:p + w])
                    nc.sync.dma_start(outT_r[:, m, n0 + t:n0 + t + w],
                                      o_sb[:, t:t + w])

        def body():
            for i, (n0, nw, slc) in enumerate(chunk_list()):
                do_chunk(n0, nw, slc, i == 0)

        if hw_loop:
            with tc.For_i(0, hw_loop, 1):
                body()
        else:
            body()
    nc.finalize()
    return nc


def build_nc_v6(C, hw_loop=0):
    """Uniform 2-slice chunks + few big DMAs + 3-deep PSUM pool.

    TimelineSim showed two costs the 1536-chunk builds pay: (a) each
    dma_start costs ~650ns of serial issue on the sync queue, so per-k /
    per-stripe descriptor spam delays the first matmul by ~8us; (b) with
    2x3-bank PSUM tiles the silu/mul round-trip doesn't fit the 2-buffer
    recycle window for narrow slices, stalling PE ~0.4us per group.

    Here: chunks are ceil(C/1024) near-equal sizes, each 2 bank-aligned
    slices -> PSUM tiles are [128,1024] (2 banks) and the pool holds 3
    bufs (6 banks): two full groups of recycle slack. Weights live in two
    monolithic SBUF tiles so each 512-col stripe (all k-tiles) is ONE
    descriptor, ordered gate-half, up-half, down; x streams one descriptor
    per slice.
    """
    nc = bacc.Bacc("TRN2", target_bir_lowering=False, debug=False, num_devices=E)
    dt = mybir.dt
    NBC = 1024
    xT = nc.dram_tensor("xT", [D, C], dt.float16, kind="ExternalInput")
    wgu = nc.dram_tensor("wguT", [D, F2], dt.float16, kind="ExternalInput")
    wd = nc.dram_tensor("wdT", [F, D], dt.float16, kind="ExternalInput")
    outT = nc.dram_tensor("outT", [D, C], dt.float32, kind="ExternalOutput")

    with TileContext(nc) as tc, ExitStack() as ctx:
        wpool = ctx.enter_context(tc.tile_pool(name="weights", bufs=1))
        wgu_sb = wpool.tile([128, KD, F2], dt.float16, tag="wgu")
        wd_sb = wpool.tile([128, KF, D], dt.float16, tag="wd")

        xpool = ctx.enter_context(tc.tile_pool(name="x", bufs=2))
        ghpool = ctx.enter_context(tc.tile_pool(name="gh", bufs=1))
        opool = ctx.enter_context(tc.tile_pool(name="o", bufs=3))
        pspool = ctx.enter_context(tc.tile_pool(name="ps", bufs=3, space="PSUM"))

        xT_r = xT.rearrange("(k p) t -> p k t", p=128)
        wgu_r = wgu.rearrange("(k p) f -> p k f", p=128)
        wd_r = wd.rearrange("(k p) m -> p k m", p=128)
        outT_r = outT.rearrange("(m p) t -> p m t", p=128)

        def chunk_list():
            """[(n0, nw, [(tok_off, psum_off, w), ...])] near-equal 2-slice
            chunks."""
            nch = -(-C // NBC)
            base, ext = divmod(C, nch)
            sizes = [base + (1 if i < ext else 0) for i in range(nch)]
            out, n0 = [], 0
            for nw in sizes:
                w0 = (nw + 1) // 2
                slc = [(0, 0, w0)]
                if nw - w0:
                    slc.append((w0, NT, nw - w0))
                out.append((n0, nw, slc))
                n0 += nw
            return out

        def emit_weight_dmas():
            # gate half first, leading 512 cols in two 256-col pieces so the
            # first matmul group unblocks after ~0.5 MB
            for c0, c1 in [(0, 256), (256, 512)] + [
                    (c, c + 512) for c in range(512, F, 512)]:
                nc.sync.dma_start(wgu_sb[:, :, c0:c1], wgu_r[:, :, c0:c1])
            for c0 in range(F, F2, 512):
                nc.sync.dma_start(wgu_sb[:, :, c0:c0 + 512],
                                  wgu_r[:, :, c0:c0 + 512])
            for c0 in range(0, D, 512):
                nc.sync.dma_start(wd_sb[:, :, c0:c0 + 512],
                                  wd_r[:, :, c0:c0 + 512])

        def do_chunk(n0, nw, slc, first_chunk):
            x_sb = xpool.tile([128, KD, NBC], dt.float16, tag="x")
            for t, p, w in slc:
                nc.sync.dma_start(x_sb[:, :, t:t + w], xT_r[:, :, n0 + t:n0 + t + w])
                if first_chunk and t == 0:
                    emit_weight_dmas()
            gh = ghpool.tile([128, KF, NBC], dt.float16, tag="gh")
            for phase in (0, 1):  # 0: gate+silu, 1: up+mul-in-place
                for mp in range(KF):
                    ps = pspool.tile([128, NBC], dt.float32, tag="ps",
                                     name=f"ps_{n0}_{phase}_{mp}")
                    f0 = mp * 128 if phase == 0 else F + mp * 128
                    for k in range(KD):
                        for t, p, w in slc:
                            nc.tensor.matmul(
                                ps[:, p:p + w],
                                lhsT=wgu_sb[:, k, ds(f0, 128)],
                                rhs=x_sb[:, k, t:t + w],
                                start=(k == 0), stop=(k == KD - 1))
                    for t, p, w in slc:
                        if phase == 0:
                            nc.scalar.activation(
                                gh[:, mp, t:t + w], ps[:, p:p + w],
                                mybir.ActivationFunctionType.Silu)
                        else:
                            nc.vector.tensor_mul(gh[:, mp, t:t + w],
                                                 gh[:, mp, t:t + w],
                                                 ps[:, p:p + w])
            for m in range(MD):
                pso = pspool.tile([128, NBC], dt.float32, tag="ps",
                                  name=f"pso_{n0}_{m}")
                for k in range(KF):
                    for t, p, w in slc:
                        nc.tensor.matmul(
                            pso[:, p:p + w], lhsT=wd_sb[:, k, ts(m, 128)],
                            rhs=gh[:, k, t:t + w],
                            start=(k == 0), stop=(k == KF - 1))
                o_sb = opool.tile([128, NBC], dt.float32, tag="o",
                                  name=f"o_{n0}_{m}")
                for t, p, w in slc:
                    nc.vector.tensor_copy(o_sb[:, t:t + w], pso[:, p:p + w])
                    nc.sync.dma_start(outT_r[:, m, n0 + t:n0 + t + w],
                                      o_sb[:, t:t + w])

        def body():
            for i, (n0, nw, slc) in enumerate(chunk_list()):
                do_chunk(n0, nw, slc, i == 0)

        if hw_loop:
            with tc.For_i(0, hw_loop, 1):
                body()
        else:
            body()
    nc.finalize()
    return nc


def build_nc_mmonly(C, hw_loop=0):
    """Microbench: gate-phase-like pure matmul stream (resident operands).
    Per-rep predicted 2.4GHz time: C*128/2.4e9 ns. Measures real PE rate."""
    nc = bacc.Bacc("TRN2", target_bir_lowering=False, debug=False, num_devices=E)
    dt = mybir.dt
    xT = nc.dram_tensor("xT", [D, C], dt.float16, kind="ExternalInput")
    wgu = nc.dram_tensor("wguT", [D, F2], dt.float16, kind="ExternalInput")
    outT = nc.dram_tensor("outT", [D, C], dt.float32, kind="ExternalOutput")
    NBC = 512
    with TileContext(nc) as tc, ExitStack() as ctx:
        wpool = ctx.enter_context(tc.tile_pool(name="weights", bufs=1))
        wg_sb = wpool.tile([128, KD, F2], dt.float16, tag="wg")
        xpool = ctx.enter_context(tc.tile_pool(name="x", bufs=1))
        x_sb = xpool.tile([128, KD, NBC], dt.float16, tag="x")
        gpool = ctx.enter_context(tc.tile_pool(name="g", bufs=2))
        pspool = ctx.enter_context(tc.tile_pool(name="ps", bufs=4, space="PSUM"))
        nc.sync.dma_start(x_sb[:], xT.rearrange("(k p) t -> p k t", p=128)[:, :, :NBC])
        for k in range(KD):
            nc.sync.dma_start(wg_sb[:, k], wgu.rearrange("(k p) f -> p k f", p=128)[:, k])

        def body():
            # same MM count as one full v7 rep-worth of gate+up+down per
            # 512 tokens x (C/512): 384 * ceil(C/512) MMs of N=512
            for rep in range(-(-C // NBC)):
                for mp in range(KF * 2 + MD):
                    ps = pspool.tile([128, NBC], dt.float32, tag="ps",
                                     name=f"ps_{rep}_{mp}")
                    f0 = (mp * 128) % F2
                    for k in range(KD):
                        nc.tensor.matmul(
                            ps[:], lhsT=wg_sb[:, k, ds(f0, 128)],
                            rhs=x_sb[:, k, :],
                            start=(k == 0), stop=(k == KD - 1))
                    g_sb = gpool.tile([128, NBC], dt.float32, tag="g",
                                      name=f"g_{rep}_{mp}")
                    nc.scalar.activation(g_sb[:], ps[:],
                                         mybir.ActivationFunctionType.Silu)
            nc.sync.dma_start(
                outT.rearrange("(m p) t -> p m t", p=128)[:, 0, :NBC],
                g_sb[:])

        if hw_loop:
            with tc.For_i(0, hw_loop, 1):
                body()
        else:
            body()
    nc.finalize()
    return nc


def build_nc_v8(C, hw_loop=0):
    """v7 + weight streams spread across the first chunk's phases."""
    return build_nc_v7(C, hw_loop=hw_loop, spread_weights=True)


def build_nc_v9(C, hw_loop=0):
    """v7 + bf16 output (halves out-DMA bytes) + all out-DMAs on the SP
    queue (the ACT queue then carries only gate stripes at a rep/kernel
    start, removing queue-level collision between the down-phase writeback
    and the next gate-weight stream)."""
    return build_nc_v7(C, hw_loop=hw_loop, out_bf16=True, outs_on_sp=True)


def build_nc_v7(C, hw_loop=0, weights_outside=False, spread_weights=False,
                out_bf16=False, outs_on_sp=False, unroll=1):
    """Near-equal 3-slice chunks + stripe-major weight tiles.

    Design notes (from TimelineSim analysis of big/v5/v6):
    - each dma_start costs ~650ns serial issue -> few, large descriptors;
    - Tile dep-tracking uses flattened-free-dim bounding boxes -> weight
      tiles are laid out stripe-major ([128, stripe, k, cols]) so one
      stripe DMA = one exact-bbox descriptor;
    - PSUM recycle (matmul group -> silu/mul -> free) takes ~2.3us, so
      chunk slices are sized so a group is >=3us: near-equal chunks of
      ~1052 tokens, 3 bank-aligned slices each, [128,1536] psum x2 bufs;
    - gate weights stream in 256-col stripes (consumption order), up half
      and down weights in 512-col stripes;
    - first chunk's first two gate groups run slice-outer so the first
      matmul needs only slice0 of x + the first gate stripe (~1 MB).
    """
    nc = bacc.Bacc("TRN2", target_bir_lowering=False, debug=False, num_devices=E)
    dt = mybir.dt
    NB = 1536
    out_dt = dt.bfloat16 if out_bf16 else dt.float32
    xT = nc.dram_tensor("xT", [D, C], dt.float16, kind="ExternalInput")
    wgu = nc.dram_tensor("wguT", [D, F2], dt.float16, kind="ExternalInput")
    wd = nc.dram_tensor("wdT", [F, D], dt.float16, kind="ExternalInput")
    outT = nc.dram_tensor("outT", [D, C], out_dt, kind="ExternalOutput")

    GS = 256   # gate-half weight stripe width
    WS = 512   # up-half / down weight stripe width
    NGS = F // GS
    with TileContext(nc) as tc, ExitStack() as ctx:
        wpool = ctx.enter_context(tc.tile_pool(name="weights", bufs=1))
        # [128, stripe, k, cols]: one DMA per stripe with an exact bbox
        wg_sb = wpool.tile([128, NGS, KD, GS], dt.float16, tag="wg")
        wu_sb = wpool.tile([128, F // WS, KD, WS], dt.float16, tag="wu")
        wd_sb = wpool.tile([128, D // WS, KF, WS], dt.float16, tag="wd")

        xpool = ctx.enter_context(tc.tile_pool(name="x", bufs=1))
        ghpool = ctx.enter_context(tc.tile_pool(name="gh", bufs=1))
        opool = ctx.enter_context(tc.tile_pool(name="o", bufs=2))
        pspool = ctx.enter_context(tc.tile_pool(name="ps", bufs=2, space="PSUM"))

        xT_r = xT.rearrange("(k p) t -> p k t", p=128)
        wgu_r = wgu.rearrange("(k p) f -> p k f", p=128)
        wd_r = wd.rearrange("(k p) m -> p k m", p=128)
        outT_r = outT.rearrange("(m p) t -> p m t", p=128)

        def gate_w(mp):  # lhsT for gate col-tile mp (128 cols)
            f0 = mp * 128
            return wg_sb[:, f0 // GS, :, (f0 % GS):(f0 % GS) + 128]

        def up_w(mp):
            f0 = mp * 128
            return wu_sb[:, f0 // WS, :, (f0 % WS):(f0 % WS) + 128]

        def down_w(m):
            f0 = m * 128
            return wd_sb[:, f0 // WS, :, (f0 % WS):(f0 % WS) + 128]

        def chunk_list():
            nch = max(1, -(-C // NB))
            base, ext = divmod(C, nch)
            sizes = [base + (1 if i < ext else 0) for i in range(nch)]
            out, n0 = [], 0
            for nw in sizes:
                ns = min(3, -(-nw // NT))
                wv, we = divmod(nw, ns)
                widths = [wv + (1 if i < we else 0) for i in range(ns)]
                slc, t = [], 0
                for i, w in enumerate(widths):
                    slc.append((t, i * NT, w))
                    t += w
                out.append((n0, nw, slc))
                n0 += nw
            return out

        def emit_gate_dmas():
            # gate stripes issue on the Activation HWDGE queue, everything
            # else on SP: the two queues issue descriptors in parallel
            # (~650ns serial issue each), so the gate path isn't starved.
            for s in range(NGS):
                nc.scalar.dma_start(wg_sb[:, s], wgu_r[:, :, s * GS:(s + 1) * GS])

        def emit_up_dmas():
            for s in range(F // WS):
                nc.sync.dma_start(wu_sb[:, s],
                                  wgu_r[:, :, F + s * WS:F + (s + 1) * WS])

        def emit_down_dmas():
            for s in range(D // WS):
                nc.sync.dma_start(wd_sb[:, s], wd_r[:, :, s * WS:(s + 1) * WS])

        def emit_weight_dmas():
            emit_gate_dmas()
            emit_up_dmas()
            emit_down_dmas()

        rep = [0]

        def do_chunk(n0, nw, slc, first_chunk, last_chunk):
            uid = f"{rep[0]}_{n0}"
            x_sb = xpool.tile([128, 3, KD, NT], dt.float16, tag="x")
            for si, (t, p, w) in enumerate(slc):
                nc.sync.dma_start(x_sb[:, si, :, :w],
                                  xT_r[:, :, n0 + t:n0 + t + w])
            if first_chunk and not weights_outside:
                if spread_weights:
                    emit_gate_dmas()
                else:
                    emit_weight_dmas()
            gh = ghpool.tile([128, KF, NB], dt.float16, tag="gh")
            for phase in (0, 1):  # 0: gate+silu, 1: up+mul-in-place
                for mp in range(KF):
                    if (first_chunk and spread_weights and not weights_outside
                            and mp == KF // 2):
                        # spread the bulk weight streams: up half midway
                        # through the gate phase, down weights midway through
                        # the up phase (~14us of lead each, > the ~11.5us of
                        # data) — decongests the rep-boundary DMA burst
                        if phase == 0:
                            emit_up_dmas()
                        else:
                            emit_down_dmas()
                    ps = pspool.tile([128, NB], dt.float32, tag="ps",
                                     name=f"ps_{uid}_{phase}_{mp}")
                    wsel = gate_w(mp) if phase == 0 else up_w(mp)
                    slice_outer = first_chunk and phase == 0 and mp < 2
                    if slice_outer:
                        loop = [(k, si) for si in range(len(slc))
                                for k in range(KD)]
                    else:
                        loop = [(k, si) for k in range(KD)
                                for si in range(len(slc))]
                    for k, si in loop:
                        t, p, w = slc[si]
                        nc.tensor.matmul(
                            ps[:, p:p + w], lhsT=wsel[:, k],
                            rhs=x_sb[:, si, k, :w],
                            start=(k == 0), stop=(k == KD - 1))
                    for t, p, w in slc:
                        if phase == 0:
                            nc.scalar.activation(
                                gh[:, mp, t:t + w], ps[:, p:p + w],
                                mybir.ActivationFunctionType.Silu)
                        else:
                            nc.vector.tensor_mul(gh[:, mp, t:t + w],
                                                 gh[:, mp, t:t + w],
                                                 ps[:, p:p + w])
            for m in range(MD):
                pso = pspool.tile([128, NB], dt.float32, tag="ps",
                                  name=f"pso_{uid}_{m}")
                # last m of the last chunk runs slice-outer: slice s's
                # accumulation closes ~5us before the kernel end, so its
                # eviction+DMA pipeline under the remaining matmuls and the
                # serial tail is one ~350-token eviction.
                tail = last_chunk and m == MD - 1
                o_sb = opool.tile([128, NB], out_dt, tag="o",
                                  name=f"o_{uid}_{m}")
                if tail:
                    for si, (t, p, w) in enumerate(slc):
                        for k in range(KF):
                            nc.tensor.matmul(
                                pso[:, p:p + w], lhsT=down_w(m)[:, k],
                                rhs=gh[:, k, t:t + w],
                                start=(k == 0), stop=(k == KF - 1))
                        nc.vector.tensor_copy(o_sb[:, t:t + w], pso[:, p:p + w])
                        dge = nc.sync if (outs_on_sp or si % 2 == 0) \
                            else nc.scalar
                        dge.dma_start(outT_r[:, m, n0 + t:n0 + t + w],
                                      o_sb[:, t:t + w])
                else:
                    for k in range(KF):
                        for t, p, w in slc:
                            nc.tensor.matmul(
                                pso[:, p:p + w], lhsT=down_w(m)[:, k],
                                rhs=gh[:, k, t:t + w],
                                start=(k == 0), stop=(k == KF - 1))
                    for t, p, w in slc:
                        nc.vector.tensor_copy(o_sb[:, t:t + w], pso[:, p:p + w])
                        dge = nc.sync if (outs_on_sp or m % 2 == 0) \
                            else nc.scalar
                        dge.dma_start(outT_r[:, m, n0 + t:n0 + t + w],
                                      o_sb[:, t:t + w])

        def body():
            chunks = chunk_list()
            for i, (n0, nw, slc) in enumerate(chunks):
                do_chunk(n0, nw, slc, i == 0, i == len(chunks) - 1)

        if weights_outside:
            emit_weight_dmas()
        if hw_loop:
            with tc.For_i(0, hw_loop, 1):
                body()
        else:
            for r in range(unroll):  # unroll>1: sim-only rep-boundary probe
                rep[0] = r
                body()
    nc.finalize()
    return nc


def build_nc_v7nw(C, hw_loop=0):
    """v7 with weight DMAs hoisted out of the hw_loop (microbench: isolates
    the per-rep 12MB weight re-stream from the loop differential)."""
    return build_nc_v7(C, hw_loop=hw_loop, weights_outside=True)


# the shipped kernel variant (used by get_nc and test.py's timing loop)
BUILD = build_nc_v9


def route(x, expert_indices):
    """Sort tokens by expert; return (order, counts, capacity C)."""
    idx = np.asarray(expert_indices)
    order = np.argsort(idx, kind="stable")
    counts = np.bincount(idx, minlength=E).astype(np.int64)
    C = max(NT, int(-(-counts.max() // 8) * 8))
    return order, counts, C


def make_in_maps(x, expert_indices, gate_up_weight, down_weight):
    order, counts, C = route(x, expert_indices)
    x_sorted = np.asarray(x, dtype=np.float32)[order]
    offs = np.zeros(E + 1, dtype=np.int64)
    np.cumsum(counts, out=offs[1:])
    wguT = np.ascontiguousarray(
        np.transpose(np.asarray(gate_up_weight), (0, 2, 1))).astype(F16)
    wdT = np.ascontiguousarray(
        np.transpose(np.asarray(down_weight), (0, 2, 1))).astype(F16)
    in_maps = []
    for e in range(E):
        xe = np.zeros((C, D), dtype=np.float32)
        xe[: counts[e]] = x_sorted[offs[e]: offs[e + 1]]
        in_maps.append({
            "xT": np.ascontiguousarray(xe.T).astype(F16),
            "wguT": wguT[e],
            "wdT": wdT[e],
        })
    return in_maps, order, counts, C


def assemble_output(results, order, counts):
    T = int(counts.sum())
    out = np.empty((T, D), dtype=np.float32)
    offs = np.zeros(E + 1, dtype=np.int64)
    np.cumsum(counts, out=offs[1:])
    sorted_out = np.empty((T, D), dtype=np.float32)
    for e in range(E):
        sorted_out[offs[e]: offs[e + 1]] = results[e]["outT"].T[: counts[e]]
    out[order] = sorted_out
    return out


def kernel(x, expert_indices, gate_up_weight, down_weight):
    in_maps, order, counts, C = make_in_maps(
        x, expert_indices, gate_up_weight, down_weight)
    nc = get_nc(C)
    res = run_bass_kernel_spmd(nc, in_maps, core_ids=list(range(E)))
    return assemble_output(res.results, order, counts)

